# revision 2
# baseline (speedup 1.0000x reference)
"""Barrier_Net TRN2 kernel: 8-core data-parallel Bass/Tile implementation.

Layout strategy (host does layout/packing only, all math on device):
  - x sharded over 8 cores by agent axis (12500 each, padded to 12800 = 25*512).
  - XT: feature-major transpose of x [128, A] (rows 0..84 = features).
  - XB: barrier operand, agent-minor layout [32, A] matched to the DVE
    32x32-block transpose of the `empty` head output.
  - MLPs run as full-width fp16 matmuls with zero-padded block-diagonal
    weights (2 neighbors / 2 obstacles per matmul), fp32 PSUM accumulate.
  - DeepSet neighbor sum is folded into the phi2/obs2 matmuls (linearity),
    accumulated in a single PSUM tile.
  - barrier + final tanh in fp32.
"""
import sys, os
sys.path.insert(0, "/opt/trn_rl_repo")
import numpy as np
import concourse.bacc as bacc
import concourse.tile as tile
import concourse.mybir as mybir
from concourse.bass_utils import run_bass_kernel_spmd
from contextlib import ExitStack

F32 = mybir.dt.float32
F16 = mybir.dt.float16
AF = mybir.ActivationFunctionType
ALU = mybir.AluOpType

B, NN, NO, SD = 100000, 16, 8, 4
H, PHI_OUT, ADIM = 64, 16, 2
DS, B_GAMMA = 0.2, 0.01
D_OBS = 85
NCORE = 8
AC = B // NCORE            # 12500 agents per core
G512 = 25                  # groups of 512
AP_ = G512 * 512           # padded agents per core = 12800
NBJ = 16                   # 32-agent blocks per group


def _pack_weights(phi_w1, phi_b1, phi_w2, phi_b2, obs_w1, obs_b1, obs_w2, obs_b2,
                  rho_w1, rho_b1, rho_w2, rho_b2, psi_w1, psi_b1, psi_w2, psi_b2):
    # phi1: 8 matmuls, each covers neighbors (2k, 2k+1); lhsT [128,128]
    W1P = np.zeros((8, 128, 128), np.float32)
    for k in range(8):
        for j in range(2):
            n = 2 * k + j
            W1P[k, 5 + 4 * n:5 + 4 * n + 4, 64 * j:64 * j + 64] = phi_w1
    W2D = np.zeros((128, 16), np.float32)
    W2D[0:64] = phi_w2
    W2D[64:128] = phi_w2
    OW1P = np.zeros((4, 128, 128), np.float32)
    for m in range(4):
        for j in range(2):
            o = 2 * m + j
            OW1P[m, 69 + 2 * o:69 + 2 * o + 2, 64 * j:64 * j + 64] = obs_w1
    OW2D = np.zeros((128, 16), np.float32)
    OW2D[0:64] = obs_w2
    OW2D[64:128] = obs_w2
    # bias columns for relu evacuations (partition layout 64j + h)
    B1C = np.tile(phi_b1, 2).astype(np.float32).reshape(128, 1)
    OB1C = np.tile(obs_b1, 2).astype(np.float32).reshape(128, 1)
    # rho stage; fold phi2/obs2 biases (summed over elements) into rho1 input
    rin_bias = NN * phi_b2 + NO * obs_b2
    RB1C = (rho_b1 + rin_bias @ rho_w1).astype(np.float32).reshape(64, 1)
    PB1C = (psi_b1 + rho_b2 @ psi_w1[0:2]).astype(np.float32).reshape(64, 1)
    PB2C = psi_b2.astype(np.float32).reshape(2, 1)
    return dict(W1P=W1P, W2D=W2D, OW1P=OW1P, OW2D=OW2D, B1C=B1C, OB1C=OB1C,
                RW1=rho_w1.astype(np.float32), RW2=rho_w2.astype(np.float32),
                RB1C=RB1C, PW1A=psi_w1[0:2].astype(np.float32),
                PW1B=psi_w1[2:4].astype(np.float32),
                PW2=psi_w2.astype(np.float32), PB1C=PB1C, PB2C=PB2C)


def _build(nc):
    xt_d = nc.dram_tensor("xt", [128, AP_], F32, kind="ExternalInput").ap()
    g_d = nc.dram_tensor("g", [2, AP_], F32, kind="ExternalInput").ap()
    xbx_d = nc.dram_tensor("xbx", [32, AP_ // 2], F32, kind="ExternalInput").ap()
    xby_d = nc.dram_tensor("xby", [32, AP_ // 2], F32, kind="ExternalInput").ap()
    w1p_d = nc.dram_tensor("w1p", [128, 8 * 128], F16, kind="ExternalInput").ap()
    w2d_d = nc.dram_tensor("w2d", [128, 16], F16, kind="ExternalInput").ap()
    ow1p_d = nc.dram_tensor("ow1p", [128, 4 * 128], F16, kind="ExternalInput").ap()
    ow2d_d = nc.dram_tensor("ow2d", [128, 16], F16, kind="ExternalInput").ap()
    small_d = nc.dram_tensor("small", [128, 8], F32, kind="ExternalInput").ap()
    # small cols: 0 B1C,1 OB1C,2 RB1C(64),3 PB1C(64),4 PB2C(2)
    rw1_d = nc.dram_tensor("rw1", [16, 64], F16, kind="ExternalInput").ap()
    rw2_d = nc.dram_tensor("rw2", [64, 2], F16, kind="ExternalInput").ap()
    pw1a_d = nc.dram_tensor("pw1a", [2, 64], F16, kind="ExternalInput").ap()
    pw1b_d = nc.dram_tensor("pw1b", [2, 64], F16, kind="ExternalInput").ap()
    pw2_d = nc.dram_tensor("pw2", [64, 2], F16, kind="ExternalInput").ap()
    y_d = nc.dram_tensor("y", [32, G512 * 32], F32, kind="ExternalOutput").ap()

    with tile.TileContext(nc) as tc, ExitStack() as ctx:
        cw = ctx.enter_context(tc.tile_pool(name="cw", bufs=1))
        xin = ctx.enter_context(tc.tile_pool(name="xin", bufs=3))
        ev = ctx.enter_context(tc.tile_pool(name="ev", bufs=3))
        sm = ctx.enter_context(tc.tile_pool(name="sm", bufs=3))
        p1 = ctx.enter_context(tc.tile_pool(name="p1", bufs=2, space="PSUM"))
        p2 = ctx.enter_context(tc.tile_pool(name="p2", bufs=2, space="PSUM"))
        p3 = ctx.enter_context(tc.tile_pool(name="p3", bufs=2, space="PSUM"))

        w1p = cw.tile([128, 8 * 128], F16); nc.sync.dma_start(w1p[:], w1p_d)
        w2dt = cw.tile([128, 16], F16); nc.sync.dma_start(w2dt[:], w2d_d)
        ow1p = cw.tile([128, 4 * 128], F16); nc.sync.dma_start(ow1p[:], ow1p_d)
        ow2dt = cw.tile([128, 16], F16); nc.sync.dma_start(ow2dt[:], ow2d_d)
        smallt = cw.tile([128, 8], F32); nc.sync.dma_start(smallt[:], small_d)
        rw1t = cw.tile([16, 64], F16); nc.sync.dma_start(rw1t[:], rw1_d)
        rw2t = cw.tile([64, 2], F16); nc.sync.dma_start(rw2t[:], rw2_d)
        pw1at = cw.tile([2, 64], F16); nc.sync.dma_start(pw1at[:], pw1a_d)
        pw1bt = cw.tile([2, 64], F16); nc.sync.dma_start(pw1bt[:], pw1b_d)
        pw2t = cw.tile([64, 2], F16); nc.sync.dma_start(pw2t[:], pw2_d)

        for gi in range(G512):
            cs = gi * 512
            xt = xin.tile([128, 512], F16)
            nc.gpsimd.dma_start(xt[:], xt_d[:, cs:cs + 512])      # cast f32->f16
            gt = xin.tile([2, 512], F16)
            nc.gpsimd.dma_start(gt[:], g_d[:, cs:cs + 512])
            xbx = xin.tile([32, 256], F32, tag="xbx")
            nc.sync.dma_start(xbx[:], xbx_d[:, gi * 256:gi * 256 + 256])
            xby = xin.tile([32, 256], F32, tag="xby")
            nc.sync.dma_start(xby[:], xby_d[:, gi * 256:gi * 256 + 256])

            # ---- deepset accumulate psum ----
            ph2 = p2.tile([16, 512], F32)
            nmm = 12
            mmi = 0
            for k in range(8):
                ps = p1.tile([128, 512], F32, tag="p1")
                nc.tensor.matmul(ps[:, :], lhsT=w1p[:, 128 * k:128 * k + 128],
                                 rhs=xt[:, :], start=True, stop=True)
                s = ev.tile([128, 512], F16, tag="s")
                if k % 2 == 0:
                    nc.scalar.activation(s[:], ps[:], AF.Relu, bias=smallt[:, 0:1])
                else:
                    nc.vector.tensor_scalar(s[:], ps[:], smallt[:, 0:1], 0.0,
                                            op0=ALU.add, op1=ALU.max)
                nc.tensor.matmul(ph2[:, :], lhsT=w2dt[:, :], rhs=s[:, :],
                                 start=(mmi == 0), stop=(mmi == nmm - 1))
                mmi += 1
            for m in range(4):
                ps = p1.tile([128, 512], F32, tag="p1")
                nc.tensor.matmul(ps[:, :], lhsT=ow1p[:, 128 * m:128 * m + 128],
                                 rhs=xt[:, :], start=True, stop=True)
                s = ev.tile([128, 512], F16, tag="s")
                if m % 2 == 0:
                    nc.scalar.activation(s[:], ps[:], AF.Relu, bias=smallt[:, 1:2])
                else:
                    nc.vector.tensor_scalar(s[:], ps[:], smallt[:, 1:2], 0.0,
                                            op0=ALU.add, op1=ALU.max)
                nc.tensor.matmul(ph2[:, :], lhsT=ow2dt[:, :], rhs=s[:, :],
                                 start=(mmi == 0), stop=(mmi == nmm - 1))
                mmi += 1

            rin = sm.tile([16, 512], F16, tag="rin")
            nc.vector.tensor_copy(rin[:], ph2[:])
            prh = p3.tile([64, 512], F32, tag="p3s")
            nc.tensor.matmul(prh[:, :], lhsT=rw1t[:, :], rhs=rin[:, :],
                             start=True, stop=True)
            rh = sm.tile([64, 512], F16, tag="rh")
            nc.scalar.activation(rh[:], prh[:], AF.Relu, bias=smallt[0:64, 2:3])
            pr2 = p3.tile([2, 512], F32, tag="p3s")
            nc.tensor.matmul(pr2[:, :], lhsT=rw2t[:, :], rhs=rh[:, :],
                             start=True, stop=True)
            r2 = sm.tile([2, 512], F16, tag="r2")
            nc.vector.tensor_copy(r2[:], pr2[:])

            pph = p3.tile([64, 512], F32, tag="p3s")
            nc.tensor.matmul(pph[:, :], lhsT=pw1at[:, :], rhs=r2[:, :],
                             start=True, stop=False)
            nc.tensor.matmul(pph[:, :], lhsT=pw1bt[:, :], rhs=gt[:, :],
                             start=False, stop=True)
            phh = sm.tile([64, 512], F16, tag="phh")
            nc.scalar.activation(phh[:], pph[:], AF.Relu, bias=smallt[0:64, 3:4])
            pe = p3.tile([2, 512], F32, tag="p3s")
            nc.tensor.matmul(pe[:, :], lhsT=pw2t[:, :], rhs=phh[:, :],
                             start=True, stop=True)
            # empty = tanh(e + pb2)
            esb = sm.tile([32, 512], F32, tag="esb")
            nc.gpsimd.memset(esb[:], 0.0)
            nc.scalar.activation(esb[0:2, :], pe[:, :], AF.Tanh,
                                 bias=smallt[0:2, 4:5])
            eT = sm.tile([32, 512], F32, tag="eT")
            nc.vector.transpose(eT[:], esb[:])

            # ---- barrier (fp32, 32 partitions) ----
            sqx = sm.tile([32, 256], F32, tag="sqx")
            nc.scalar.activation(sqx[:], xbx[:], AF.Square)
            n2 = sm.tile([32, 256], F32, tag="n2")
            sqy = sm.tile([32, 256], F32, tag="sqy")
            nc.scalar.activation(sqy[:], xby[:], AF.Square)
            nc.vector.tensor_add(n2[:], sqx[:], sqy[:])
            sr = sm.tile([32, 256], F32, tag="sr")
            nc.scalar.activation(sr[:], n2[:], AF.Sqrt)
            dd = sm.tile([32, 256], F32, tag="dd")
            nc.vector.tensor_scalar(dd[:], sr[:], -DS, 1.0 / B_GAMMA,
                                    op0=ALU.add, op1=ALU.mult)
            rr = sm.tile([32, 256], F32, tag="rr")
            nc.vector.reciprocal_approx_fast(out=rr[:], in_=dd[:])
            rpx = sm.tile([32, 256], F32, tag="rpx")
            nc.vector.tensor_mul(rpx[:], xbx[:], rr[:])
            rpy = sm.tile([32, 256], F32, tag="rpy")
            nc.vector.tensor_mul(rpy[:], xby[:], rr[:])
            barx = sm.tile([32, 16], F32, tag="barx")
            nc.vector.tensor_reduce(
                out=barx[:], in_=rpx[:].rearrange("p (b n) -> p b n", n=16),
                axis=mybir.AxisListType.X, op=ALU.add)
            bary = sm.tile([32, 16], F32, tag="bary")
            nc.vector.tensor_reduce(
                out=bary[:], in_=rpy[:].rearrange("p (b n) -> p b n", n=16),
                axis=mybir.AxisListType.X, op=ALU.add)
            act = sm.tile([32, 32], F32, tag="act")
            eTr = eT[:].rearrange("p (b w) -> p b w", w=32)
            actr = act[:].rearrange("p (b u) -> p b u", u=2)
            nc.vector.tensor_add(actr[:, :, 0:1], eTr[:, :, 0:1],
                                 barx[:].rearrange("p (b o) -> p b o", o=1))
            nc.vector.tensor_add(actr[:, :, 1:2], eTr[:, :, 1:2],
                                 bary[:].rearrange("p (b o) -> p b o", o=1))
            yt = sm.tile([32, 32], F32, tag="yt")
            nc.scalar.activation(yt[:], act[:], AF.Tanh)
            yt2 = sm.tile([32, 32], F32, tag="yt2")
            nc.vector.tensor_scalar_mul(yt2[:], yt[:], 2.0)
            nc.sync.dma_start(y_d[:, gi * 32:gi * 32 + 32], yt2[:])
    return nc


_CACHED = {}


def kernel(**inputs):
    x = np.asarray(inputs["x"], np.float32)
    wk = _pack_weights(**{k: np.asarray(v, np.float32) for k, v in inputs.items()
                          if k != "x"})
    const = {
        "w1p": wk["W1P"].transpose(1, 0, 2).reshape(128, 8 * 128).astype(np.float16),
        "w2d": wk["W2D"].astype(np.float16),
        "ow1p": wk["OW1P"].transpose(1, 0, 2).reshape(128, 4 * 128).astype(np.float16),
        "ow2d": wk["OW2D"].astype(np.float16),
        "rw1": wk["RW1"].astype(np.float16), "rw2": wk["RW2"].astype(np.float16),
        "pw1a": wk["PW1A"].astype(np.float16), "pw1b": wk["PW1B"].astype(np.float16),
        "pw2": wk["PW2"].astype(np.float16),
    }
    small = np.zeros((128, 8), np.float32)
    small[:, 0:1] = wk["B1C"]; small[:, 1:2] = wk["OB1C"]
    small[0:64, 2:3] = wk["RB1C"]; small[0:64, 3:4] = wk["PB1C"]
    small[0:2, 4:5] = wk["PB2C"]
    const["small"] = small

    in_maps = []
    for c in range(NCORE):
        xs = x[c * AC:(c + 1) * AC]
        xp = np.zeros((AP_, D_OBS), np.float32)
        xp[:AC] = xs
        xt = np.zeros((128, AP_), np.float32)
        xt[0:D_OBS] = xp.T
        g = np.ascontiguousarray(xp[:, 0:2].T)
        # XB: partition a, col gi*512 + bj*32 + (2*jn + u) = -nb[idx, jn, u]
        # agent idx = gi*512 + bj*32 + a
        xr = xp[:, 5:69].reshape(AP_, 16, 4)[:, :, 0:2]      # [A,16,2] = nb
        # p_ij = -nb; cols per (gi,bj): 16 neighbors
        px = (-xr[:, :, 0]).reshape(G512, NBJ, 32, 16).transpose(2, 0, 1, 3)
        py = (-xr[:, :, 1]).reshape(G512, NBJ, 32, 16).transpose(2, 0, 1, 3)
        m = dict(const)
        m["xt"] = np.ascontiguousarray(xt)
        m["g"] = np.ascontiguousarray(g)
        m["xbx"] = np.ascontiguousarray(px.reshape(32, AP_ // 2))
        m["xby"] = np.ascontiguousarray(py.reshape(32, AP_ // 2))
        in_maps.append(m)

    if "nc" not in _CACHED:
        nc = bacc.Bacc("TRN2", target_bir_lowering=False, debug=False,
                       num_devices=NCORE)
        _build(nc)
        nc.compile()
        _CACHED["nc"] = nc
    nc = _CACHED["nc"]
    res = run_bass_kernel_spmd(nc, in_maps, core_ids=list(range(NCORE)))
    _CACHED["last_res"] = res
    out = np.empty((B, ADIM), np.float32)
    for c in range(NCORE):
        Y = res.results[c]["y"]                              # [32, G512*32]
        Y4 = Y.reshape(32, G512, NBJ, 2).transpose(1, 2, 0, 3).reshape(AP_, 2)
        out[c * AC:(c + 1) * AC] = Y4[:AC]
    return out


if __name__ == "__main__":
    import reference
    ins = {k: np.asarray(v) for k, v in reference.setup_inputs().items()}
    got = kernel(**ins)
    exp = np.asarray(reference.reference(**ins))
    err = np.abs(got - exp).max()
    rel = err / np.abs(exp).max()
    print(f"absmax {err:.4e} rel {rel:.4e}")



# revision 9
# speedup vs baseline: 1.9122x; 1.9122x over previous
"""Barrier_Net TRN2 kernel: 8-core data-parallel Bass/Tile implementation (v2).

Per-core structure (12800 padded agents = 25 groups x 512):
  - Layer-1 MLPs: 12 fp16 matmuls/group (weights pre-scaled x32), relu evac
    to fp8e4 (values 32*relu) split across ScalarE/VectorE.
  - Layer-2 + rho1 fused by linearity (lhsT = (w2 @ rho_w1) * 8 in fp8),
    6 DoubleRow fp8 matmuls/group (contract 256) accumulate rho hidden
    pre-act (x256) in one PSUM tile.
  - rho2 + psi1 fused (lhsT rows 0:64 = rho_w2 @ psi_w1[:2] / 256), g rows
    appended to the same rhs tile -> single matmul.
  - empty-head e stashed per group into a [32,512] accumulator (16 groups),
    transposed once per chunk, tanh'd in batch (one act-table switch total).
  - barrier computed fully in fp32 (accuracy-critical) in a [128,1600]x2
    layout, neighbor reduce on DVE + partition fold via matmul with ones.
"""
import sys, os
sys.path.insert(0, "/opt/trn_rl_repo")
import numpy as np
import ml_dtypes
import concourse.bacc as bacc
import concourse.tile as tile
import concourse.mybir as mybir
from concourse.bass_utils import run_bass_kernel_spmd
from contextlib import ExitStack

F32 = mybir.dt.float32
F16 = mybir.dt.float16
F8 = mybir.dt.float8e4
AF = mybir.ActivationFunctionType
ALU = mybir.AluOpType
PM = mybir.MatmulPerfMode

B, NN, NO, SD = 100000, 16, 8, 4
H, PHI_OUT, ADIM = 64, 16, 2
DS, B_GAMMA = 0.2, 0.01
D_OBS = 85
NCORE = 8
AC = B // NCORE            # 12500 agents per core
G512 = 25                  # groups of 512
AP_ = G512 * 512           # padded agents per core = 12800
S1 = 32.0                  # layer-1 weight prescale
S2 = 8.0                   # fused layer-2 weight prescale
BARC = G512 * 16 * 4       # 1600 barrier cols per half

SCALAR_EVACS = frozenset({0, 1, 3, 4, 6, 7, 9, 10})


def _pack_weights(phi_w1, phi_b1, phi_w2, phi_b2, obs_w1, obs_b1, obs_w2, obs_b2,
                  rho_w1, rho_b1, rho_w2, rho_b2, psi_w1, psi_b1, psi_w2, psi_b2):
    # layer-1 block-diagonal lhsT, 2 elements per matmul, prescaled x32
    W1P = np.zeros((8, 128, 128), np.float32)
    for k in range(8):
        for j in range(2):
            n = 2 * k + j
            W1P[k, 5 + 4 * n:5 + 4 * n + 4, 64 * j:64 * j + 64] = phi_w1 * S1
    OW1P = np.zeros((4, 128, 128), np.float32)
    for m in range(4):
        for j in range(2):
            o = 2 * m + j
            OW1P[m, 69 + 2 * o:69 + 2 * o + 2, 64 * j:64 * j + 64] = obs_w1 * S1
    # fused layer-2 + rho1 (DoubleRow lhsT [128, ko=2, 64]), x8
    W2R = np.tile((phi_w2 @ rho_w1) * S2, (2, 1))       # [128, 64]
    OW2R = np.tile((obs_w2 @ rho_w1) * S2, (2, 1))
    l2w = np.zeros((128, 2, 2, 64), np.float32)          # [p, which, ko, j]
    l2w[:, 0, 0] = l2w[:, 0, 1] = W2R
    l2w[:, 1, 0] = l2w[:, 1, 1] = OW2R
    # fused rho2+psi1 lhsT [66, 64]; rh carries a 1/(S1*S2) scale already
    PW = np.zeros((66, 64), np.float32)
    PW[0:64] = rho_w2 @ psi_w1[0:2]
    PW[64:66] = psi_w1[2:4]
    # bias columns
    RB1C = (rho_b1 + (NN * phi_b2 + NO * obs_b2) @ rho_w1).reshape(64, 1)
    PB1C = (psi_b1 + rho_b2 @ psi_w1[0:2]).reshape(64, 1)
    PB2C = psi_b2.reshape(2, 1)
    small = np.zeros((128, 8), np.float32)
    small[:, 0:1] = np.tile(phi_b1, 2).reshape(128, 1) * S1
    small[:, 1:2] = np.tile(obs_b1, 2).reshape(128, 1) * S1
    small[0:64, 2:3] = RB1C
    small[0:64, 3:4] = PB1C
    small[0:2, 4:5] = PB2C
    ones4 = np.zeros((128, 32), np.float32)
    for p in range(128):
        ones4[p, p % 32] = 1.0
    f8 = ml_dtypes.float8_e4m3
    return {
        "w1p": W1P.transpose(1, 0, 2).reshape(128, 8 * 128).astype(np.float16),
        "ow1p": OW1P.transpose(1, 0, 2).reshape(128, 4 * 128).astype(np.float16),
        "l2w": l2w.reshape(128, 256).astype(f8),
        "pw": PW.astype(np.float16),
        "pw2": psi_w2.astype(np.float16),
        "ones4": ones4.astype(np.float16),
        "small": small.astype(np.float32),
    }


def _build(nc):
    xt_d = nc.dram_tensor("xt", [128, AP_], F16, kind="ExternalInput").ap()
    gg_d = nc.dram_tensor("gg", [2, AP_], F16, kind="ExternalInput").ap()
    xb_d = nc.dram_tensor("xb", [128, 2 * BARC], F32, kind="ExternalInput").ap()
    w1p_d = nc.dram_tensor("w1p", [128, 8 * 128], F16, kind="ExternalInput").ap()
    ow1p_d = nc.dram_tensor("ow1p", [128, 4 * 128], F16, kind="ExternalInput").ap()
    l2w_d = nc.dram_tensor("l2w", [128, 256], F8, kind="ExternalInput").ap()
    pw_d = nc.dram_tensor("pw", [66, 64], F16, kind="ExternalInput").ap()
    pw2_d = nc.dram_tensor("pw2", [64, 2], F16, kind="ExternalInput").ap()
    ones4_d = nc.dram_tensor("ones4", [128, 32], F16, kind="ExternalInput").ap()
    small_d = nc.dram_tensor("small", [128, 8], F32, kind="ExternalInput").ap()
    y_d = nc.dram_tensor("y", [32, 1024], F32, kind="ExternalOutput").ap()

    with tile.TileContext(nc) as tc, ExitStack() as ctx, \
            nc.allow_low_precision(reason="fp16 barrier partials validated"):
        cw = ctx.enter_context(tc.tile_pool(name="cw", bufs=1))
        xin = ctx.enter_context(tc.tile_pool(name="xin", bufs=3))
        s8p = ctx.enter_context(tc.tile_pool(name="s8p", bufs=2))
        rgp = ctx.enter_context(tc.tile_pool(name="rgp", bufs=2))
        pep = ctx.enter_context(tc.tile_pool(name="pep", bufs=2))
        bw = ctx.enter_context(tc.tile_pool(name="bw", bufs=1))
        p1 = ctx.enter_context(tc.tile_pool(name="p1", bufs=3, space="PSUM"))
        p2 = ctx.enter_context(tc.tile_pool(name="p2", bufs=2, space="PSUM"))
        p3 = ctx.enter_context(tc.tile_pool(name="p3", bufs=2, space="PSUM"))
        pbar = ctx.enter_context(tc.tile_pool(name="pbar", bufs=1, space="PSUM"))

        w1p = cw.tile([128, 8 * 128], F16); nc.sync.dma_start(w1p[:], w1p_d)
        ow1p = cw.tile([128, 4 * 128], F16); nc.sync.dma_start(ow1p[:], ow1p_d)
        l2w = cw.tile([128, 256], F8); nc.sync.dma_start(l2w[:], l2w_d)
        pw = cw.tile([66, 64], F16); nc.sync.dma_start(pw[:], pw_d)
        pw2 = cw.tile([64, 2], F16); nc.sync.dma_start(pw2[:], pw2_d)
        ones4 = cw.tile([128, 32], F16); nc.sync.dma_start(ones4[:], ones4_d)
        small = cw.tile([128, 8], F32); nc.sync.dma_start(small[:], small_d)
        xb = cw.tile([128, 2 * BARC], F32); nc.sync.dma_start(xb[:], xb_d)

        # barrier working tiles (fp32 until the neighbor reduce)
        sqx = bw.tile([128, BARC], F32)
        sqy = bw.tile([128, BARC], F32)
        n2 = bw.tile([128, BARC], F32)
        sr = bw.tile([128, BARC], F32)
        dd = bw.tile([128, BARC], F32)
        rr = bw.tile([128, BARC], F32)
        rpx = bw.tile([128, BARC], F32)
        rpy = bw.tile([128, BARC], F32)
        ptx = bw.tile([128, BARC // 4], F16)
        pty = bw.tile([128, BARC // 4], F16)
        barS = bw.tile([32, 1024], F32)
        xbx = xb[:, 0:BARC]
        xby = xb[:, BARC:2 * BARC]

        def bar_step(i):
            if i == 0:
                nc.vector.tensor_mul(sqx[:], xbx, xbx)
            elif i == 1:
                nc.vector.tensor_mul(sqy[:], xby, xby)
            elif i == 2:
                nc.vector.tensor_add(n2[:], sqx[:], sqy[:])
            elif i == 3:
                nc.scalar.activation(sr[:], n2[:], AF.Sqrt)
            elif i == 4:
                nc.vector.tensor_scalar(dd[:], sr[:], 1.0 / B_GAMMA,
                                        -DS / B_GAMMA, op0=ALU.mult, op1=ALU.add)
            elif i == 5:
                nc.vector.reciprocal_approx_fast(out=rr[:], in_=dd[:])
            elif i == 6:
                nc.vector.tensor_mul(rpx[:], xbx, rr[:])
            elif i == 7:
                nc.vector.tensor_mul(rpy[:], xby, rr[:])
            elif i == 8:
                nc.vector.tensor_reduce(
                    out=ptx[:], in_=rpx[:].rearrange("p (c f) -> p c f", f=4),
                    axis=mybir.AxisListType.X, op=ALU.add)
            elif i == 9:
                nc.vector.tensor_reduce(
                    out=pty[:], in_=rpy[:].rearrange("p (c f) -> p c f", f=4),
                    axis=mybir.AxisListType.X, op=ALU.add)
            elif i == 10:
                bx = pbar.tile([32, 400], F32, tag="bps")
                nc.tensor.matmul(bx[:], lhsT=ones4[:], rhs=ptx[:],
                                 start=True, stop=True)
                nc.vector.tensor_copy(barS[:, 0:400], bx[:])
            elif i == 11:
                by = pbar.tile([32, 400], F32, tag="bps")
                nc.tensor.matmul(by[:], lhsT=ones4[:], rhs=pty[:],
                                 start=True, stop=True)
                nc.vector.tensor_copy(barS[:, 512:912], by[:])

        peacc = [None, None]

        def chunk_tail(c):
            et = pep.tile([32, 512], F32, tag="et")
            nc.vector.transpose(et[:], peacc[c][:])
            e1 = pep.tile([32, 512], F32, tag="e1")
            nc.scalar.activation(e1[:], et[:], AF.Tanh)
            act = pep.tile([32, 512], F32, tag="act")
            actv = act[:].rearrange("p (b m u) -> p b m u", m=16, u=2)
            e1v = e1[:].rearrange("p (b m u) -> p b m u", m=16, u=2)
            bxv = barS[:, 256 * c:256 * c + 256].rearrange(
                "p (m b o) -> p b m o", b=16, o=1)
            byv = barS[:, 512 + 256 * c:512 + 256 * c + 256].rearrange(
                "p (m b o) -> p b m o", b=16, o=1)
            nc.vector.tensor_add(actv[:, :, :, 0:1], e1v[:, :, :, 0:1], bxv)
            nc.vector.tensor_add(actv[:, :, :, 1:2], e1v[:, :, :, 1:2], byv)
            yt = pep.tile([32, 512], F32, tag="yt")
            nc.scalar.activation(yt[:], act[:], AF.Tanh)
            y2 = pep.tile([32, 512], F32, tag="y2")
            nc.vector.tensor_scalar_mul(y2[:], yt[:], 2.0)
            nc.sync.dma_start(y_d[:, 512 * c:512 * c + 512], y2[:])

        for g in range(G512):
            cs = g * 512
            m, c = g % 16, g // 16
            xt = xin.tile([128, 512], F16, tag="xt")
            nc.sync.dma_start(xt[:], xt_d[:, cs:cs + 512])
            rg = rgp.tile([66, 512], F16, tag="rg")
            nc.sync.dma_start(rg[64:66, :], gg_d[:, cs:cs + 512])

            parents = []
            for j in range(6):
                parents.append(s8p.tile([128, 1024], F8, tag=f"s{j}",
                                        name=f"s8_{g}_{j}"))
            for idx in range(12):
                j, ko = idx // 2, idx % 2
                ps = p1.tile([128, 512], F32, tag="ps")
                if idx < 8:
                    lhsT = w1p[:, idx * 128:idx * 128 + 128]
                    bcol = small[:, 0:1]
                else:
                    lhsT = ow1p[:, (idx - 8) * 128:(idx - 8) * 128 + 128]
                    bcol = small[:, 1:2]
                nc.tensor.matmul(ps[:], lhsT=lhsT, rhs=xt[:], start=True, stop=True)
                dst = parents[j][:, ko * 512:ko * 512 + 512]
                if idx in SCALAR_EVACS:
                    nc.scalar.activation(dst, ps[:], AF.Relu, bias=bcol)
                else:
                    nc.vector.tensor_scalar(dst, ps[:], bcol, 0.0,
                                            op0=ALU.add, op1=ALU.max)

            prh = p2.tile([64, 512], F32, tag="prh")
            for j in range(6):
                rhs = parents[j][:].rearrange("p (ko n) -> p ko n", ko=2)
                wsel = l2w[:, 0:128] if j < 4 else l2w[:, 128:256]
                lhsT = wsel.rearrange("p (ko q) -> p ko q", ko=2)
                nc.tensor.matmul(prh[:], lhsT=lhsT, rhs=rhs,
                                 start=(j == 0), stop=(j == 5),
                                 perf_mode=PM.DoubleRow)
            nc.scalar.activation(rg[0:64, :], prh[:], AF.Relu,
                                 bias=small[0:64, 2:3], scale=1.0 / (S1 * S2))
            pb = p3.tile([128, 512], F32, tag="pb")
            nc.tensor.matmul(pb[0:64, :], lhsT=pw[:], rhs=rg[:], start=True, stop=True)
            phh = rgp.tile([64, 512], F16, tag="phh")
            nc.vector.tensor_scalar(phh[:], pb[0:64, :], small[0:64, 3:4], 0.0,
                                    op0=ALU.add, op1=ALU.max)
            nc.tensor.matmul(pb[64:66, :], lhsT=pw2[:], rhs=phh[:],
                             start=True, stop=True, tile_position=(0, 64))
            if m == 0:
                peacc[c] = pep.tile([32, 512], F32, tag="pe", name=f"peacc{c}")
            petmp = rgp.tile([2, 512], F32, tag="petmp")
            nc.vector.tensor_scalar_add(petmp[:], pb[64:66, :], small[0:2, 4:5])
            nc.sync.dma_start(peacc[c][2 * m:2 * m + 2, :], petmp[:])

            if g < 12:
                bar_step(g)
            if g == 15:
                chunk_tail(0)
        chunk_tail(1)
    return nc


_CACHED = {}


def kernel(**inputs):
    x = np.asarray(inputs["x"], np.float32)
    wk = _pack_weights(**{k: np.asarray(v, np.float32) for k, v in inputs.items()
                          if k != "x"})
    in_maps = []
    for core in range(NCORE):
        xp = np.zeros((AP_, D_OBS), np.float32)
        xp[:AC] = x[core * AC:(core + 1) * AC]
        xt = np.zeros((128, AP_), np.float16)
        xt[0:D_OBS] = xp.T.astype(np.float16)
        gg = np.ascontiguousarray(xp[:, 0:2].T.astype(np.float16))
        p = -xp[:, 5:69].reshape(AP_, 16, 4)[:, :, 0:2]       # [A, 16, 2]
        # [gi, bj, a32, nhi, nlo] -> partition 32*nlo + a32, col (gi*16+bj)*4+nhi
        pr = p.reshape(G512, 16, 32, 4, 4, 2)
        xbh = pr.transpose(5, 4, 2, 0, 1, 3).reshape(2, 128, BARC)
        xb = np.ascontiguousarray(
            np.concatenate([xbh[0], xbh[1]], axis=1).astype(np.float32))
        m = dict(wk)
        m["xt"] = np.ascontiguousarray(xt)
        m["gg"] = gg
        m["xb"] = xb
        in_maps.append(m)

    if "nc" not in _CACHED:
        nc = bacc.Bacc("TRN2", target_bir_lowering=False, debug=False,
                       num_devices=NCORE)
        _build(nc)
        nc.compile()
        _CACHED["nc"] = nc
    nc = _CACHED["nc"]
    res = run_bass_kernel_spmd(nc, in_maps, core_ids=list(range(NCORE)))
    _CACHED["last_res"] = res
    out = np.empty((B, ADIM), np.float32)
    for core in range(NCORE):
        Y = res.results[core]["y"]                            # [32, 1024]
        Y5 = Y.reshape(32, 2, 16, 16, 2).transpose(1, 3, 2, 0, 4)
        Y5 = Y5.reshape(32, 512, 2)[:G512].reshape(AP_, 2)
        out[core * AC:(core + 1) * AC] = Y5[:AC]
    return out


if __name__ == "__main__":
    import reference
    ins = {k: np.asarray(v) for k, v in reference.setup_inputs().items()}
    got = kernel(**ins)
    exp = np.asarray(reference.reference(**ins))
    err = np.abs(got - exp).max()
    rel = err / np.abs(exp).max()
    print(f"absmax {err:.4e} rel {rel:.4e}")


# revision 12
# speedup vs baseline: 2.0579x; 1.0762x over previous
"""Barrier_Net TRN2 kernel: 8-core data-parallel Bass/Tile implementation (v2).

Per-core structure (12800 padded agents = 25 groups x 512):
  - Layer-1 MLPs: 12 fp16 matmuls/group (weights pre-scaled x32), relu evac
    to fp8e4 (values 32*relu) split across ScalarE/VectorE.
  - Layer-2 + rho1 fused by linearity (lhsT = (w2 @ rho_w1) * 8 in fp8),
    6 DoubleRow fp8 matmuls/group (contract 256) accumulate rho hidden
    pre-act (x256) in one PSUM tile.
  - rho2 + psi1 fused (lhsT rows 0:64 = rho_w2 @ psi_w1[:2] / 256), g rows
    appended to the same rhs tile -> single matmul.
  - empty-head e stashed per group into a [32,512] accumulator (16 groups),
    transposed once per chunk, tanh'd in batch (one act-table switch total).
  - barrier computed fully in fp32 (accuracy-critical) in a [128,1600]x2
    layout, neighbor reduce on DVE + partition fold via matmul with ones.
"""
import sys, os
sys.path.insert(0, "/opt/trn_rl_repo")
import numpy as np
import ml_dtypes
import concourse.bacc as bacc
import concourse.tile as tile
import concourse.mybir as mybir
from concourse.bass_utils import run_bass_kernel_spmd
from contextlib import ExitStack

F32 = mybir.dt.float32
F16 = mybir.dt.float16
F8 = mybir.dt.float8e4
AF = mybir.ActivationFunctionType
ALU = mybir.AluOpType
PM = mybir.MatmulPerfMode

B, NN, NO, SD = 100000, 16, 8, 4
H, PHI_OUT, ADIM = 64, 16, 2
DS, B_GAMMA = 0.2, 0.01
D_OBS = 85
NCORE = 8
AC = B // NCORE            # 12500 agents per core
G512 = 25                  # groups of 512
AP_ = G512 * 512           # padded agents per core = 12800
S1 = 32.0                  # layer-1 weight prescale
S2 = 8.0                   # fused layer-2 weight prescale
BARC = G512 * 16 * 4       # 1600 barrier cols per half

SCALAR_EVACS = frozenset({0, 1, 3, 4, 6, 7, 9, 10})


def _pack_weights(phi_w1, phi_b1, phi_w2, phi_b2, obs_w1, obs_b1, obs_w2, obs_b2,
                  rho_w1, rho_b1, rho_w2, rho_b2, psi_w1, psi_b1, psi_w2, psi_b2):
    # layer-1 block-diagonal lhsT, 2 elements per matmul, prescaled x32
    W1P = np.zeros((8, 128, 128), np.float32)
    for k in range(8):
        for j in range(2):
            n = 2 * k + j
            W1P[k, 5 + 4 * n:5 + 4 * n + 4, 64 * j:64 * j + 64] = phi_w1 * S1
    OW1P = np.zeros((4, 128, 128), np.float32)
    for m in range(4):
        for j in range(2):
            o = 2 * m + j
            OW1P[m, 69 + 2 * o:69 + 2 * o + 2, 64 * j:64 * j + 64] = obs_w1 * S1
    # fused layer-2 + rho1 (DoubleRow lhsT [128, ko=2, 64]), x8
    W2R = np.tile((phi_w2 @ rho_w1) * S2, (2, 1))       # [128, 64]
    OW2R = np.tile((obs_w2 @ rho_w1) * S2, (2, 1))
    l2w = np.zeros((128, 2, 2, 64), np.float32)          # [p, which, ko, j]
    l2w[:, 0, 0] = l2w[:, 0, 1] = W2R
    l2w[:, 1, 0] = l2w[:, 1, 1] = OW2R
    # fused rho2+psi1 lhsT [66, 64]; rh carries a 1/(S1*S2) scale already
    PW = np.zeros((66, 64), np.float32)
    PW[0:64] = rho_w2 @ psi_w1[0:2]
    PW[64:66] = psi_w1[2:4]
    # bias columns
    RB1C = (rho_b1 + (NN * phi_b2 + NO * obs_b2) @ rho_w1).reshape(64, 1)
    PB1C = (psi_b1 + rho_b2 @ psi_w1[0:2]).reshape(64, 1)
    PB2C = psi_b2.reshape(2, 1)
    small = np.zeros((128, 8), np.float32)
    small[:, 0:1] = np.tile(phi_b1, 2).reshape(128, 1) * S1
    small[:, 1:2] = np.tile(obs_b1, 2).reshape(128, 1) * S1
    small[0:64, 2:3] = RB1C
    small[0:64, 3:4] = PB1C
    small[0:2, 4:5] = PB2C
    ones4 = np.zeros((128, 32), np.float32)
    for p in range(128):
        ones4[p, p % 32] = 1.0
    f8 = ml_dtypes.float8_e4m3
    return {
        "w1p": W1P.transpose(1, 0, 2).reshape(128, 8 * 128).astype(np.float16),
        "ow1p": OW1P.transpose(1, 0, 2).reshape(128, 4 * 128).astype(np.float16),
        "l2w": l2w.reshape(128, 256).astype(f8),
        "pw": PW.astype(np.float16),
        "pw2": psi_w2.astype(np.float16),
        "ones4": ones4.astype(np.float16),
        "small": small.astype(np.float32),
    }


def _build(nc):
    xt_d = nc.dram_tensor("xt", [128, AP_], F16, kind="ExternalInput").ap()
    gg_d = nc.dram_tensor("gg", [2, AP_], F16, kind="ExternalInput").ap()
    xb_d = nc.dram_tensor("xb", [128, 2 * BARC], F32, kind="ExternalInput").ap()
    w1p_d = nc.dram_tensor("w1p", [128, 8 * 128], F16, kind="ExternalInput").ap()
    ow1p_d = nc.dram_tensor("ow1p", [128, 4 * 128], F16, kind="ExternalInput").ap()
    l2w_d = nc.dram_tensor("l2w", [128, 256], F8, kind="ExternalInput").ap()
    pw_d = nc.dram_tensor("pw", [66, 64], F16, kind="ExternalInput").ap()
    pw2_d = nc.dram_tensor("pw2", [64, 2], F16, kind="ExternalInput").ap()
    ones4_d = nc.dram_tensor("ones4", [128, 32], F16, kind="ExternalInput").ap()
    small_d = nc.dram_tensor("small", [128, 8], F32, kind="ExternalInput").ap()
    y_d = nc.dram_tensor("y", [32, 1024], F32, kind="ExternalOutput").ap()

    with tile.TileContext(nc) as tc, ExitStack() as ctx, \
            nc.allow_low_precision(reason="fp16 barrier partials validated"):
        cw = ctx.enter_context(tc.tile_pool(name="cw", bufs=1))
        xin = ctx.enter_context(tc.tile_pool(name="xin", bufs=3))
        s8p = ctx.enter_context(tc.tile_pool(name="s8p", bufs=2))
        rgp = ctx.enter_context(tc.tile_pool(name="rgp", bufs=2))
        pep = ctx.enter_context(tc.tile_pool(name="pep", bufs=2))
        bw = ctx.enter_context(tc.tile_pool(name="bw", bufs=1))
        p1 = ctx.enter_context(tc.tile_pool(name="p1", bufs=3, space="PSUM"))
        p2 = ctx.enter_context(tc.tile_pool(name="p2", bufs=1, space="PSUM"))
        p3 = ctx.enter_context(tc.tile_pool(name="p3", bufs=1, space="PSUM"))

        w1p = cw.tile([128, 8 * 128], F16); nc.sync.dma_start(w1p[:], w1p_d)
        ow1p = cw.tile([128, 4 * 128], F16); nc.sync.dma_start(ow1p[:], ow1p_d)
        l2w = cw.tile([128, 256], F8); nc.sync.dma_start(l2w[:], l2w_d)
        pw = cw.tile([66, 64], F16); nc.sync.dma_start(pw[:], pw_d)
        pw2 = cw.tile([64, 2], F16); nc.sync.dma_start(pw2[:], pw2_d)
        ones4 = cw.tile([128, 32], F16); nc.sync.dma_start(ones4[:], ones4_d)
        small = cw.tile([128, 8], F32); nc.sync.dma_start(small[:], small_d)
        xb = cw.tile([128, 2 * BARC], F32); nc.sync.dma_start(xb[:], xb_d)

        # barrier working tiles (fp32 until the neighbor reduce)
        sqx = bw.tile([128, BARC], F32)
        sqy = bw.tile([128, BARC], F32)
        n2 = bw.tile([128, BARC], F32)
        sr = bw.tile([128, BARC], F32)
        dd = bw.tile([128, BARC], F32)
        rr = bw.tile([128, BARC], F32)
        rpx = bw.tile([128, BARC], F32)
        rpy = bw.tile([128, BARC], F32)
        ptx = bw.tile([128, BARC // 4], F16)
        pty = bw.tile([128, BARC // 4], F16)
        barS = bw.tile([32, 1024], F32)
        xbx = xb[:, 0:BARC]
        xby = xb[:, BARC:2 * BARC]

        def bar_step(i):
            if i == 0:
                nc.vector.tensor_mul(sqx[:], xbx, xbx)
            elif i == 1:
                nc.vector.tensor_mul(sqy[:], xby, xby)
            elif i == 2:
                nc.vector.tensor_add(n2[:], sqx[:], sqy[:])
            elif i == 3:
                nc.scalar.activation(sr[:], n2[:], AF.Sqrt)
            elif i == 4:
                nc.vector.tensor_scalar(dd[:], sr[:], 1.0 / B_GAMMA,
                                        -DS / B_GAMMA, op0=ALU.mult, op1=ALU.add)
            elif i == 5:
                nc.vector.reciprocal_approx_fast(out=rr[:], in_=dd[:])
            elif i == 6:
                nc.vector.tensor_mul(rpx[:], xbx, rr[:])
            elif i == 7:
                nc.vector.tensor_mul(rpy[:], xby, rr[:])
            elif i == 8:
                nc.vector.tensor_reduce(
                    out=ptx[:], in_=rpx[:].rearrange("p (c f) -> p c f", f=4),
                    axis=mybir.AxisListType.X, op=ALU.add)
            elif i == 9:
                nc.vector.tensor_reduce(
                    out=pty[:], in_=rpy[:].rearrange("p (c f) -> p c f", f=4),
                    axis=mybir.AxisListType.X, op=ALU.add)
            elif i == 10:
                bx = p3.tile([32, 400], F32, tag="pb", name="barxp")
                nc.tensor.matmul(bx[:], lhsT=ones4[:], rhs=ptx[:],
                                 start=True, stop=True)
                nc.vector.tensor_copy(barS[:, 0:400], bx[:])
            elif i == 11:
                by = p3.tile([32, 400], F32, tag="pb", name="baryp")
                nc.tensor.matmul(by[:], lhsT=ones4[:], rhs=pty[:],
                                 start=True, stop=True)
                nc.vector.tensor_copy(barS[:, 512:912], by[:])

        peacc = [None, None]

        def chunk_tail(c):
            et = pep.tile([32, 512], F32, tag="et")
            nc.vector.transpose(et[:], peacc[c][:])
            e1 = pep.tile([32, 512], F32, tag="e1")
            nc.scalar.activation(e1[:], et[:], AF.Tanh)
            act = pep.tile([32, 512], F32, tag="act")
            actv = act[:].rearrange("p (b m u) -> p b m u", m=16, u=2)
            e1v = e1[:].rearrange("p (b m u) -> p b m u", m=16, u=2)
            bxv = barS[:, 256 * c:256 * c + 256].rearrange(
                "p (m b o) -> p b m o", b=16, o=1)
            byv = barS[:, 512 + 256 * c:512 + 256 * c + 256].rearrange(
                "p (m b o) -> p b m o", b=16, o=1)
            nc.vector.tensor_add(actv[:, :, :, 0:1], e1v[:, :, :, 0:1], bxv)
            nc.vector.tensor_add(actv[:, :, :, 1:2], e1v[:, :, :, 1:2], byv)
            yt = pep.tile([32, 512], F32, tag="yt")
            nc.scalar.activation(yt[:], act[:], AF.Tanh)
            y2 = pep.tile([32, 512], F32, tag="y2")
            nc.vector.tensor_scalar_mul(y2[:], yt[:], 2.0)
            nc.sync.dma_start(y_d[:, 512 * c:512 * c + 512], y2[:])

        # 3-stage software pipeline: iteration i runs layer-1(i) interleaved
        # with the fused layer-2 DR matmuls of group i-1 and the psi head of
        # group i-2, so TensorE never waits on a just-issued evac.
        st1 = None   # group i-1 state
        st2 = None   # group i-2 state
        for i in range(G512 + 2):
            cur = None
            if i < G512:
                cs = i * 512
                xt = xin.tile([128, 512], F16, tag="xt", name=f"xt{i}")
                nc.sync.dma_start(xt[:], xt_d[:, cs:cs + 512])
                rg = rgp.tile([66, 512], F16, tag="rg", bufs=3, name=f"rg{i}")
                nc.sync.dma_start(rg[64:66, :], gg_d[:, cs:cs + 512])
                parents = [s8p.tile([128, 1024], F8, tag=f"s{j}",
                                    name=f"s8_{i}_{j}") for j in range(6)]
                cur = dict(i=i, xt=xt, rg=rg, parents=parents)

            if st2 is not None:
                # psi head for group i-2 (rh/phh evacs ran last iteration)
                g2 = st2["i"]
                m2, c2 = g2 % 16, g2 // 16
                pb = p3.tile([128, 512], F32, tag="pb", name=f"pb{g2}")
                nc.tensor.matmul(pb[0:64, :], lhsT=pw[:], rhs=st2["rg"][:],
                                 start=True, stop=True)

            prh = None
            if st1 is not None:
                prh = p2.tile([64, 512], F32, tag="prh", name=f"prh{st1['i']}")
                st1["prh"] = prh

            for j in range(6):
                if cur is not None:
                    ps = p1.tile([128, 1024], F32, tag="ps", name=f"ps_{i}_{j}")
                    for ko in range(2):
                        idx = 2 * j + ko
                        if idx < 8:
                            lhsT = w1p[:, idx * 128:idx * 128 + 128]
                        else:
                            lhsT = ow1p[:, (idx - 8) * 128:(idx - 8) * 128 + 128]
                        nc.tensor.matmul(ps[:, ko * 512:ko * 512 + 512],
                                         lhsT=lhsT, rhs=cur["xt"][:],
                                         start=True, stop=True)
                    bcol = small[:, 0:1] if j < 4 else small[:, 1:2]
                    dst = cur["parents"][j][:]
                    if j % 2 == 0:
                        nc.scalar.activation(dst, ps[:], AF.Relu, bias=bcol)
                    else:
                        nc.vector.tensor_scalar(dst, ps[:], bcol, 0.0,
                                                op0=ALU.add, op1=ALU.max)
                if st1 is not None:
                    rhs = st1["parents"][j][:].rearrange("p (ko n) -> p ko n", ko=2)
                    wsel = l2w[:, 0:128] if j < 4 else l2w[:, 128:256]
                    lhsT = wsel.rearrange("p (ko q) -> p ko q", ko=2)
                    nc.tensor.matmul(st1["prh"][:], lhsT=lhsT, rhs=rhs,
                                     start=(j == 0), stop=(j == 5),
                                     perf_mode=PM.DoubleRow)

            if st2 is not None:
                phh = rgp.tile([64, 512], F16, tag="phh", name=f"phh{g2}")
                nc.vector.tensor_scalar(phh[:], pb[0:64, :], small[0:64, 3:4],
                                        0.0, op0=ALU.add, op1=ALU.max)
                nc.tensor.matmul(pb[64:66, :], lhsT=pw2[:], rhs=phh[:],
                                 start=True, stop=True, tile_position=(0, 64))
                if m2 == 0:
                    peacc[c2] = pep.tile([32, 512], F32, tag="pe",
                                         name=f"peacc{c2}")
                petmp = rgp.tile([2, 512], F32, tag="petmp", name=f"petmp{g2}")
                nc.vector.tensor_scalar_add(petmp[:], pb[64:66, :],
                                            small[0:2, 4:5])
                nc.gpsimd.dma_start(peacc[c2][2 * m2:2 * m2 + 2, :], petmp[:])

            if st1 is not None:
                nc.scalar.activation(st1["rg"][0:64, :], st1["prh"][:], AF.Relu,
                                     bias=small[0:64, 2:3], scale=1.0 / (S1 * S2))

            if i < 12:
                bar_step(i)
            if st2 is not None and st2["i"] == 15:
                chunk_tail(0)
            st2, st1 = st1, cur
        chunk_tail(1)
    return nc


_CACHED = {}


def kernel(**inputs):
    x = np.asarray(inputs["x"], np.float32)
    wk = _pack_weights(**{k: np.asarray(v, np.float32) for k, v in inputs.items()
                          if k != "x"})
    in_maps = []
    for core in range(NCORE):
        xp = np.zeros((AP_, D_OBS), np.float32)
        xp[:AC] = x[core * AC:(core + 1) * AC]
        xt = np.zeros((128, AP_), np.float16)
        xt[0:D_OBS] = xp.T.astype(np.float16)
        gg = np.ascontiguousarray(xp[:, 0:2].T.astype(np.float16))
        p = -xp[:, 5:69].reshape(AP_, 16, 4)[:, :, 0:2]       # [A, 16, 2]
        # [gi, bj, a32, nhi, nlo] -> partition 32*nlo + a32, col (gi*16+bj)*4+nhi
        pr = p.reshape(G512, 16, 32, 4, 4, 2)
        xbh = pr.transpose(5, 4, 2, 0, 1, 3).reshape(2, 128, BARC)
        xb = np.ascontiguousarray(
            np.concatenate([xbh[0], xbh[1]], axis=1).astype(np.float32))
        m = dict(wk)
        m["xt"] = np.ascontiguousarray(xt)
        m["gg"] = gg
        m["xb"] = xb
        in_maps.append(m)

    if "nc" not in _CACHED:
        nc = bacc.Bacc("TRN2", target_bir_lowering=False, debug=False,
                       num_devices=NCORE)
        _build(nc)
        nc.compile()
        _CACHED["nc"] = nc
    nc = _CACHED["nc"]
    res = run_bass_kernel_spmd(nc, in_maps, core_ids=list(range(NCORE)))
    _CACHED["last_res"] = res
    out = np.empty((B, ADIM), np.float32)
    for core in range(NCORE):
        Y = res.results[core]["y"]                            # [32, 1024]
        Y5 = Y.reshape(32, 2, 16, 16, 2).transpose(1, 3, 2, 0, 4)
        Y5 = Y5.reshape(32, 512, 2)[:G512].reshape(AP_, 2)
        out[core * AC:(core + 1) * AC] = Y5[:AC]
    return out


if __name__ == "__main__":
    import reference
    ins = {k: np.asarray(v) for k, v in reference.setup_inputs().items()}
    got = kernel(**ins)
    exp = np.asarray(reference.reference(**ins))
    err = np.abs(got - exp).max()
    rel = err / np.abs(exp).max()
    print(f"absmax {err:.4e} rel {rel:.4e}")


# revision 14
# speedup vs baseline: 2.1677x; 1.0534x over previous
"""Barrier_Net TRN2 kernel: 8-core data-parallel Bass/Tile implementation (v2).

Per-core structure (12800 padded agents = 25 groups x 512):
  - Layer-1 MLPs: 12 fp16 matmuls/group (weights pre-scaled x32), relu evac
    to fp8e4 (values 32*relu) split across ScalarE/VectorE.
  - Layer-2 + rho1 fused by linearity (lhsT = (w2 @ rho_w1) * 8 in fp8),
    6 DoubleRow fp8 matmuls/group (contract 256) accumulate rho hidden
    pre-act (x256) in one PSUM tile.
  - rho2 + psi1 fused (lhsT rows 0:64 = rho_w2 @ psi_w1[:2] / 256), g rows
    appended to the same rhs tile -> single matmul.
  - empty-head e stashed per group into a [32,512] accumulator (16 groups),
    transposed once per chunk, tanh'd in batch (one act-table switch total).
  - barrier computed fully in fp32 (accuracy-critical) in a [128,1600]x2
    layout, neighbor reduce on DVE + partition fold via matmul with ones.
"""
import sys, os
sys.path.insert(0, "/opt/trn_rl_repo")
import numpy as np
import ml_dtypes
import concourse.bacc as bacc
import concourse.tile as tile
import concourse.mybir as mybir
from concourse.bass_utils import run_bass_kernel_spmd
from contextlib import ExitStack

F32 = mybir.dt.float32
F16 = mybir.dt.float16
F8 = mybir.dt.float8e4
AF = mybir.ActivationFunctionType
ALU = mybir.AluOpType
PM = mybir.MatmulPerfMode

B, NN, NO, SD = 100000, 16, 8, 4
H, PHI_OUT, ADIM = 64, 16, 2
DS, B_GAMMA = 0.2, 0.01
D_OBS = 85
NCORE = 8
AC = B // NCORE            # 12500 agents per core
G512 = 25                  # groups of 512
AP_ = G512 * 512           # padded agents per core = 12800
S1 = 32.0                  # layer-1 weight prescale
S2 = 8.0                   # fused layer-2 weight prescale
BARC = G512 * 16 * 4       # 1600 barrier cols per half

SCALAR_EVACS = frozenset({0, 1, 3, 4, 6, 7, 9, 10})


def _pack_weights(phi_w1, phi_b1, phi_w2, phi_b2, obs_w1, obs_b1, obs_w2, obs_b2,
                  rho_w1, rho_b1, rho_w2, rho_b2, psi_w1, psi_b1, psi_w2, psi_b2):
    # layer-1 block-diagonal lhsT, 2 elements per matmul, prescaled x32
    W1P = np.zeros((8, 128, 128), np.float32)
    for k in range(8):
        for j in range(2):
            n = 2 * k + j
            W1P[k, 5 + 4 * n:5 + 4 * n + 4, 64 * j:64 * j + 64] = phi_w1 * S1
    OW1P = np.zeros((4, 128, 128), np.float32)
    for m in range(4):
        for j in range(2):
            o = 2 * m + j
            OW1P[m, 69 + 2 * o:69 + 2 * o + 2, 64 * j:64 * j + 64] = obs_w1 * S1
    # fused layer-2 + rho1 (DoubleRow lhsT [128, ko=2, 64]), x8
    W2R = np.tile((phi_w2 @ rho_w1) * S2, (2, 1))       # [128, 64]
    OW2R = np.tile((obs_w2 @ rho_w1) * S2, (2, 1))
    l2w = np.zeros((128, 2, 2, 64), np.float32)          # [p, which, ko, j]
    l2w[:, 0, 0] = l2w[:, 0, 1] = W2R
    l2w[:, 1, 0] = l2w[:, 1, 1] = OW2R
    # fused rho2+psi1 lhsT [66, 64]; rh carries a 1/(S1*S2) scale already
    PW = np.zeros((66, 64), np.float32)
    PW[0:64] = rho_w2 @ psi_w1[0:2]
    PW[64:66] = psi_w1[2:4]
    # bias columns
    RB1C = (rho_b1 + (NN * phi_b2 + NO * obs_b2) @ rho_w1).reshape(64, 1)
    PB1C = (psi_b1 + rho_b2 @ psi_w1[0:2]).reshape(64, 1)
    PB2C = psi_b2.reshape(2, 1)
    small = np.zeros((128, 8), np.float32)
    small[:, 0:1] = np.tile(phi_b1, 2).reshape(128, 1) * S1
    small[:, 1:2] = np.tile(obs_b1, 2).reshape(128, 1) * S1
    small[0:64, 2:3] = RB1C
    small[0:64, 3:4] = PB1C
    small[0:2, 4:5] = PB2C
    ones4 = np.zeros((128, 32), np.float32)
    for p in range(128):
        ones4[p, p % 32] = 1.0
    f8 = ml_dtypes.float8_e4m3
    PW2B = np.concatenate([psi_w2, psi_b2.reshape(1, 2)], axis=0)
    return {
        "w1p": W1P.transpose(1, 0, 2).reshape(128, 8 * 128).astype(np.float16),
        "ow1p": OW1P.transpose(1, 0, 2).reshape(128, 4 * 128).astype(np.float16),
        "l2w": l2w.reshape(128, 256).astype(f8),
        "pw": PW.astype(np.float16),
        "pw2": PW2B.astype(np.float16),
        "ones4": ones4.astype(np.float16),
        "small": small.astype(np.float32),
    }


def _build(nc):
    xt_d = nc.dram_tensor("xt", [128, AP_], F16, kind="ExternalInput").ap()
    gg_d = nc.dram_tensor("gg", [2, AP_], F16, kind="ExternalInput").ap()
    xb_d = nc.dram_tensor("xb", [128, 2 * BARC], F32, kind="ExternalInput").ap()
    w1p_d = nc.dram_tensor("w1p", [128, 8 * 128], F16, kind="ExternalInput").ap()
    ow1p_d = nc.dram_tensor("ow1p", [128, 4 * 128], F16, kind="ExternalInput").ap()
    l2w_d = nc.dram_tensor("l2w", [128, 256], F8, kind="ExternalInput").ap()
    pw_d = nc.dram_tensor("pw", [66, 64], F16, kind="ExternalInput").ap()
    pw2_d = nc.dram_tensor("pw2", [65, 2], F16, kind="ExternalInput").ap()
    ones4_d = nc.dram_tensor("ones4", [128, 32], F16, kind="ExternalInput").ap()
    small_d = nc.dram_tensor("small", [128, 8], F32, kind="ExternalInput").ap()
    y_d = nc.dram_tensor("y", [32, 1024], F32, kind="ExternalOutput").ap()

    with tile.TileContext(nc) as tc, ExitStack() as ctx, \
            nc.allow_low_precision(reason="fp16 barrier partials validated"):
        cw = ctx.enter_context(tc.tile_pool(name="cw", bufs=1))
        xin = ctx.enter_context(tc.tile_pool(name="xin", bufs=3))
        s8p = ctx.enter_context(tc.tile_pool(name="s8p", bufs=2))
        rgp = ctx.enter_context(tc.tile_pool(name="rgp", bufs=2))
        pep = ctx.enter_context(tc.tile_pool(name="pep", bufs=2))
        bw = ctx.enter_context(tc.tile_pool(name="bw", bufs=1))
        p1 = ctx.enter_context(tc.tile_pool(name="p1", bufs=3, space="PSUM"))
        p2 = ctx.enter_context(tc.tile_pool(name="p2", bufs=1, space="PSUM"))
        p3 = ctx.enter_context(tc.tile_pool(name="p3", bufs=1, space="PSUM"))

        w1p = cw.tile([128, 8 * 128], F16); nc.sync.dma_start(w1p[:], w1p_d)
        ow1p = cw.tile([128, 4 * 128], F16); nc.sync.dma_start(ow1p[:], ow1p_d)
        l2w = cw.tile([128, 256], F8); nc.sync.dma_start(l2w[:], l2w_d)
        pw = cw.tile([66, 64], F16); nc.sync.dma_start(pw[:], pw_d)
        pw2 = cw.tile([65, 2], F16); nc.sync.dma_start(pw2[:], pw2_d)
        ones4 = cw.tile([128, 32], F16); nc.sync.dma_start(ones4[:], ones4_d)
        small = cw.tile([128, 8], F32); nc.sync.dma_start(small[:], small_d)
        xb = cw.tile([128, 2 * BARC], F32); nc.sync.dma_start(xb[:], xb_d)

        # barrier working tiles (fp32 until the neighbor reduce)
        sqx = bw.tile([128, BARC], F32)
        sqy = bw.tile([128, BARC], F32)
        n2 = bw.tile([128, BARC], F32)
        sr = bw.tile([128, BARC], F32)
        dd = bw.tile([128, BARC], F32)
        rr = bw.tile([128, BARC], F32)
        rpx = bw.tile([128, BARC], F32)
        rpy = bw.tile([128, BARC], F32)
        ptx = bw.tile([128, BARC // 4], F16)
        pty = bw.tile([128, BARC // 4], F16)
        barS = bw.tile([32, 1024], F32)
        xbx = xb[:, 0:BARC]
        xby = xb[:, BARC:2 * BARC]

        def bar_step(i):
            if i == 0:
                nc.gpsimd.tensor_tensor(sqx[:], xbx, xbx, op=ALU.mult)
            elif i == 1:
                nc.gpsimd.tensor_tensor(sqy[:], xby, xby, op=ALU.mult)
            elif i == 2:
                nc.gpsimd.tensor_tensor(n2[:], sqx[:], sqy[:], op=ALU.add)
            elif i == 3:
                nc.scalar.activation(sr[:], n2[:], AF.Sqrt)
            elif i == 4:
                nc.vector.tensor_scalar(dd[:], sr[:], 1.0 / B_GAMMA,
                                        -DS / B_GAMMA, op0=ALU.mult, op1=ALU.add)
            elif i == 5:
                nc.vector.reciprocal_approx_fast(out=rr[:], in_=dd[:])
            elif i == 6:
                nc.vector.tensor_mul(rpx[:], xbx, rr[:])
            elif i == 7:
                nc.vector.tensor_mul(rpy[:], xby, rr[:])
            elif i == 8:
                nc.vector.tensor_reduce(
                    out=ptx[:], in_=rpx[:].rearrange("p (c f) -> p c f", f=4),
                    axis=mybir.AxisListType.X, op=ALU.add)
            elif i == 9:
                nc.vector.tensor_reduce(
                    out=pty[:], in_=rpy[:].rearrange("p (c f) -> p c f", f=4),
                    axis=mybir.AxisListType.X, op=ALU.add)
            elif i == 10:
                bx = p3.tile([32, 400], F32, tag="pb", name="barxp")
                nc.tensor.matmul(bx[:], lhsT=ones4[:], rhs=ptx[:],
                                 start=True, stop=True)
                nc.vector.tensor_copy(barS[:, 0:400], bx[:])
            elif i == 11:
                by = p3.tile([32, 400], F32, tag="pb", name="baryp")
                nc.tensor.matmul(by[:], lhsT=ones4[:], rhs=pty[:],
                                 start=True, stop=True)
                nc.vector.tensor_copy(barS[:, 512:912], by[:])

        peacc = [None, None]

        def chunk_tail(c):
            et = pep.tile([32, 512], F32, tag="et")
            nc.vector.transpose(et[:], peacc[c][:])
            e1 = pep.tile([32, 512], F32, tag="e1")
            nc.scalar.activation(e1[:], et[:], AF.Tanh)
            act = pep.tile([32, 512], F32, tag="act")
            actv = act[:].rearrange("p (b m u) -> p b m u", m=16, u=2)
            e1v = e1[:].rearrange("p (b m u) -> p b m u", m=16, u=2)
            bxv = barS[:, 256 * c:256 * c + 256].rearrange(
                "p (m b o) -> p b m o", b=16, o=1)
            byv = barS[:, 512 + 256 * c:512 + 256 * c + 256].rearrange(
                "p (m b o) -> p b m o", b=16, o=1)
            nc.vector.tensor_add(actv[:, :, :, 0:1], e1v[:, :, :, 0:1], bxv)
            nc.vector.tensor_add(actv[:, :, :, 1:2], e1v[:, :, :, 1:2], byv)
            yt = pep.tile([32, 512], F32, tag="yt")
            nc.scalar.activation(yt[:], act[:], AF.Tanh)
            y2 = pep.tile([32, 512], F32, tag="y2")
            nc.vector.tensor_scalar_mul(y2[:], yt[:], 2.0)
            nc.sync.dma_start(y_d[:, 512 * c:512 * c + 512], y2[:])

        # 3-stage software pipeline: iteration i runs layer-1(i) interleaved
        # with the fused layer-2 DR matmuls of group i-1 and the psi head of
        # group i-2, so TensorE never waits on a just-issued evac.
        st1 = None   # group i-1 state
        st2 = None   # group i-2 state
        for i in range(G512 + 2):
            cur = None
            if i < G512:
                cs = i * 512
                xt = xin.tile([128, 512], F16, tag="xt", name=f"xt{i}")
                nc.sync.dma_start(xt[:], xt_d[:, cs:cs + 512])
                rg = rgp.tile([66, 512], F16, tag="rg", bufs=3, name=f"rg{i}")
                nc.sync.dma_start(rg[64:66, :], gg_d[:, cs:cs + 512])
                parents = [s8p.tile([128, 1024], F8, tag=f"s{j}",
                                    name=f"s8_{i}_{j}") for j in range(6)]
                cur = dict(i=i, xt=xt, rg=rg, parents=parents)

            if st2 is not None:
                # psi head for group i-2 (rh/phh evacs ran last iteration)
                g2 = st2["i"]
                m2, c2 = g2 % 16, g2 // 16
                pb = p3.tile([128, 512], F32, tag="pb", name=f"pb{g2}")
                nc.tensor.matmul(pb[0:64, :], lhsT=pw[:], rhs=st2["rg"][:],
                                 start=True, stop=True)

            prh = None
            if st1 is not None:
                prh = p2.tile([64, 512], F32, tag="prh", name=f"prh{st1['i']}")
                st1["prh"] = prh

            for j in range(6):
                if cur is not None:
                    ps = p1.tile([128, 1024], F32, tag="ps", name=f"ps_{i}_{j}")
                    for ko in range(2):
                        idx = 2 * j + ko
                        if idx < 8:
                            lhsT = w1p[:, idx * 128:idx * 128 + 128]
                        else:
                            lhsT = ow1p[:, (idx - 8) * 128:(idx - 8) * 128 + 128]
                        nc.tensor.matmul(ps[:, ko * 512:ko * 512 + 512],
                                         lhsT=lhsT, rhs=cur["xt"][:],
                                         start=True, stop=True)
                    bcol = small[:, 0:1] if j < 4 else small[:, 1:2]
                    dst = cur["parents"][j][:]
                    if j % 2 == 0:
                        nc.scalar.activation(dst, ps[:], AF.Relu, bias=bcol)
                    else:
                        nc.vector.tensor_scalar(dst, ps[:], bcol, 0.0,
                                                op0=ALU.add, op1=ALU.max)
                if st1 is not None:
                    rhs = st1["parents"][j][:].rearrange("p (ko n) -> p ko n", ko=2)
                    wsel = l2w[:, 0:128] if j < 4 else l2w[:, 128:256]
                    lhsT = wsel.rearrange("p (ko q) -> p ko q", ko=2)
                    nc.tensor.matmul(st1["prh"][:], lhsT=lhsT, rhs=rhs,
                                     start=(j == 0), stop=(j == 5),
                                     perf_mode=PM.DoubleRow)

            if st2 is not None:
                phh = rgp.tile([65, 512], F16, tag="phh", name=f"phh{g2}")
                nc.gpsimd.memset(phh[64:65, :], 1.0)
                nc.scalar.activation(phh[0:64, :], pb[0:64, :], AF.Relu,
                                     bias=small[0:64, 3:4])
                nc.tensor.matmul(pb[64:66, :], lhsT=pw2[:], rhs=phh[:],
                                 start=True, stop=True, tile_position=(0, 64))
                if m2 == 0:
                    peacc[c2] = pep.tile([32, 512], F32, tag="pe",
                                         name=f"peacc{c2}")
                petmp = rgp.tile([2, 512], F32, tag="petmp", name=f"petmp{g2}")
                nc.vector.tensor_copy(petmp[:], pb[64:66, :])
                nc.gpsimd.dma_start(peacc[c2][2 * m2:2 * m2 + 2, :], petmp[:])

            if st1 is not None:
                nc.scalar.activation(st1["rg"][0:64, :], st1["prh"][:], AF.Relu,
                                     bias=small[0:64, 2:3], scale=1.0 / (S1 * S2))

            if i < 12:
                bar_step(i)
            if st2 is not None and st2["i"] == 15:
                chunk_tail(0)
            st2, st1 = st1, cur
        chunk_tail(1)
    return nc


_CACHED = {}


def kernel(**inputs):
    x = np.asarray(inputs["x"], np.float32)
    wk = _pack_weights(**{k: np.asarray(v, np.float32) for k, v in inputs.items()
                          if k != "x"})
    in_maps = []
    for core in range(NCORE):
        xp = np.zeros((AP_, D_OBS), np.float32)
        xp[:AC] = x[core * AC:(core + 1) * AC]
        xt = np.zeros((128, AP_), np.float16)
        xt[0:D_OBS] = xp.T.astype(np.float16)
        gg = np.ascontiguousarray(xp[:, 0:2].T.astype(np.float16))
        p = -xp[:, 5:69].reshape(AP_, 16, 4)[:, :, 0:2]       # [A, 16, 2]
        # [gi, bj, a32, nhi, nlo] -> partition 32*nlo + a32, col (gi*16+bj)*4+nhi
        pr = p.reshape(G512, 16, 32, 4, 4, 2)
        xbh = pr.transpose(5, 4, 2, 0, 1, 3).reshape(2, 128, BARC)
        xb = np.ascontiguousarray(
            np.concatenate([xbh[0], xbh[1]], axis=1).astype(np.float32))
        m = dict(wk)
        m["xt"] = np.ascontiguousarray(xt)
        m["gg"] = gg
        m["xb"] = xb
        in_maps.append(m)

    if "nc" not in _CACHED:
        nc = bacc.Bacc("TRN2", target_bir_lowering=False, debug=False,
                       num_devices=NCORE)
        _build(nc)
        nc.compile()
        _CACHED["nc"] = nc
    nc = _CACHED["nc"]
    res = run_bass_kernel_spmd(nc, in_maps, core_ids=list(range(NCORE)))
    _CACHED["last_res"] = res
    out = np.empty((B, ADIM), np.float32)
    for core in range(NCORE):
        Y = res.results[core]["y"]                            # [32, 1024]
        Y5 = Y.reshape(32, 2, 16, 16, 2).transpose(1, 3, 2, 0, 4)
        Y5 = Y5.reshape(32, 512, 2)[:G512].reshape(AP_, 2)
        out[core * AC:(core + 1) * AC] = Y5[:AC]
    return out


if __name__ == "__main__":
    import reference
    ins = {k: np.asarray(v) for k, v in reference.setup_inputs().items()}
    got = kernel(**ins)
    exp = np.asarray(reference.reference(**ins))
    err = np.abs(got - exp).max()
    rel = err / np.abs(exp).max()
    print(f"absmax {err:.4e} rel {rel:.4e}")


# revision 15
# speedup vs baseline: 2.2424x; 1.0344x over previous
"""Barrier_Net TRN2 kernel: 8-core data-parallel Bass/Tile implementation (v2).

Per-core structure (12800 padded agents = 25 groups x 512):
  - Layer-1 MLPs: 12 fp16 matmuls/group (weights pre-scaled x32), relu evac
    to fp8e4 (values 32*relu) split across ScalarE/VectorE.
  - Layer-2 + rho1 fused by linearity (lhsT = (w2 @ rho_w1) * 8 in fp8),
    6 DoubleRow fp8 matmuls/group (contract 256) accumulate rho hidden
    pre-act (x256) in one PSUM tile.
  - rho2 + psi1 fused (lhsT rows 0:64 = rho_w2 @ psi_w1[:2] / 256), g rows
    appended to the same rhs tile -> single matmul.
  - empty-head e stashed per group into a [32,512] accumulator (16 groups),
    transposed once per chunk, tanh'd in batch (one act-table switch total).
  - barrier computed fully in fp32 (accuracy-critical) in a [128,1600]x2
    layout, neighbor reduce on DVE + partition fold via matmul with ones.
"""
import sys, os
sys.path.insert(0, "/opt/trn_rl_repo")
import numpy as np
import ml_dtypes
import concourse.bacc as bacc
import concourse.tile as tile
import concourse.mybir as mybir
from concourse.bass_utils import run_bass_kernel_spmd
from contextlib import ExitStack

F32 = mybir.dt.float32
F16 = mybir.dt.float16
F8 = mybir.dt.float8e4
AF = mybir.ActivationFunctionType
ALU = mybir.AluOpType
PM = mybir.MatmulPerfMode

B, NN, NO, SD = 100000, 16, 8, 4
H, PHI_OUT, ADIM = 64, 16, 2
DS, B_GAMMA = 0.2, 0.01
D_OBS = 85
NCORE = 8
AC = B // NCORE            # 12500 agents per core
G512 = 25                  # groups of 512
AP_ = G512 * 512           # padded agents per core = 12800
S1 = 32.0                  # layer-1 weight prescale
S2 = 8.0                   # fused layer-2 weight prescale
BARC = G512 * 16 * 4       # 1600 barrier cols per half

SCALAR_EVACS = frozenset({0, 1, 3, 4, 6, 7, 9, 10})


def _pack_weights(phi_w1, phi_b1, phi_w2, phi_b2, obs_w1, obs_b1, obs_w2, obs_b2,
                  rho_w1, rho_b1, rho_w2, rho_b2, psi_w1, psi_b1, psi_w2, psi_b2):
    # layer-1 block-diagonal lhsT, 2 elements per matmul, prescaled x32
    W1P = np.zeros((8, 128, 128), np.float32)
    for k in range(8):
        for j in range(2):
            n = 2 * k + j
            W1P[k, 5 + 4 * n:5 + 4 * n + 4, 64 * j:64 * j + 64] = phi_w1 * S1
    OW1P = np.zeros((4, 128, 128), np.float32)
    for m in range(4):
        for j in range(2):
            o = 2 * m + j
            OW1P[m, 69 + 2 * o:69 + 2 * o + 2, 64 * j:64 * j + 64] = obs_w1 * S1
    # fused layer-2 + rho1 (DoubleRow lhsT [128, ko=2, 64]), x8
    W2R = np.tile((phi_w2 @ rho_w1) * S2, (2, 1))       # [128, 64]
    OW2R = np.tile((obs_w2 @ rho_w1) * S2, (2, 1))
    l2w = np.zeros((128, 2, 2, 64), np.float32)          # [p, which, ko, j]
    l2w[:, 0, 0] = l2w[:, 0, 1] = W2R
    l2w[:, 1, 0] = l2w[:, 1, 1] = OW2R
    # fused rho2+psi1 lhsT [66, 64]; rh carries a 1/(S1*S2) scale already
    PW = np.zeros((66, 64), np.float32)
    PW[0:64] = rho_w2 @ psi_w1[0:2]
    PW[64:66] = psi_w1[2:4]
    # bias columns
    RB1C = (rho_b1 + (NN * phi_b2 + NO * obs_b2) @ rho_w1).reshape(64, 1)
    PB1C = (psi_b1 + rho_b2 @ psi_w1[0:2]).reshape(64, 1)
    PB2C = psi_b2.reshape(2, 1)
    small = np.zeros((128, 8), np.float32)
    small[:, 0:1] = np.tile(phi_b1, 2).reshape(128, 1) * S1
    small[:, 1:2] = np.tile(obs_b1, 2).reshape(128, 1) * S1
    small[0:64, 2:3] = RB1C
    small[0:64, 3:4] = PB1C
    small[0:2, 4:5] = PB2C
    ones4 = np.zeros((128, 32), np.float32)
    for p in range(128):
        ones4[p, p % 32] = 1.0
    f8 = ml_dtypes.float8_e4m3
    PW2B = np.concatenate([psi_w2, psi_b2.reshape(1, 2)], axis=0)
    return {
        "w1p": W1P.transpose(1, 0, 2).reshape(128, 8 * 128).astype(np.float16),
        "ow1p": OW1P.transpose(1, 0, 2).reshape(128, 4 * 128).astype(np.float16),
        "l2w": l2w.reshape(128, 256).astype(f8),
        "pw": PW.astype(np.float16),
        "pw2": PW2B.astype(np.float16),
        "ones4": ones4.astype(np.float16),
        "small": small.astype(np.float32),
    }


def _build(nc):
    xt_d = nc.dram_tensor("xt", [128, AP_], F16, kind="ExternalInput").ap()
    gg_d = nc.dram_tensor("gg", [2, AP_], F16, kind="ExternalInput").ap()
    xb_d = nc.dram_tensor("xb", [128, 2 * BARC], F32, kind="ExternalInput").ap()
    w1p_d = nc.dram_tensor("w1p", [128, 8 * 128], F16, kind="ExternalInput").ap()
    ow1p_d = nc.dram_tensor("ow1p", [128, 4 * 128], F16, kind="ExternalInput").ap()
    l2w_d = nc.dram_tensor("l2w", [128, 256], F8, kind="ExternalInput").ap()
    pw_d = nc.dram_tensor("pw", [66, 64], F16, kind="ExternalInput").ap()
    pw2_d = nc.dram_tensor("pw2", [65, 2], F16, kind="ExternalInput").ap()
    ones4_d = nc.dram_tensor("ones4", [128, 32], F16, kind="ExternalInput").ap()
    small_d = nc.dram_tensor("small", [128, 8], F32, kind="ExternalInput").ap()
    y_d = nc.dram_tensor("y", [32, 1024], F32, kind="ExternalOutput").ap()

    with tile.TileContext(nc) as tc, ExitStack() as ctx, \
            nc.allow_low_precision(reason="fp16 barrier partials validated"):
        cw = ctx.enter_context(tc.tile_pool(name="cw", bufs=1))
        xin = ctx.enter_context(tc.tile_pool(name="xin", bufs=3))
        s8p = ctx.enter_context(tc.tile_pool(name="s8p", bufs=2))
        rgp = ctx.enter_context(tc.tile_pool(name="rgp", bufs=2))
        pep = ctx.enter_context(tc.tile_pool(name="pep", bufs=2))
        bw = ctx.enter_context(tc.tile_pool(name="bw", bufs=1))
        p1 = ctx.enter_context(tc.tile_pool(name="p1", bufs=3, space="PSUM"))
        p2 = ctx.enter_context(tc.tile_pool(name="p2", bufs=1, space="PSUM"))
        p3 = ctx.enter_context(tc.tile_pool(name="p3", bufs=1, space="PSUM"))

        w1p = cw.tile([128, 8 * 128], F16); nc.sync.dma_start(w1p[:], w1p_d)
        ow1p = cw.tile([128, 4 * 128], F16); nc.sync.dma_start(ow1p[:], ow1p_d)
        l2w = cw.tile([128, 256], F8); nc.sync.dma_start(l2w[:], l2w_d)
        pw = cw.tile([66, 64], F16); nc.sync.dma_start(pw[:], pw_d)
        pw2 = cw.tile([65, 2], F16); nc.sync.dma_start(pw2[:], pw2_d)
        ones4 = cw.tile([128, 32], F16); nc.sync.dma_start(ones4[:], ones4_d)
        small = cw.tile([128, 8], F32); nc.sync.dma_start(small[:], small_d)
        xb = cw.tile([128, 2 * BARC], F32); nc.gpsimd.dma_start(xb[:], xb_d)

        # barrier working tiles (fp32 until the neighbor reduce)
        sqx = bw.tile([128, BARC], F32)
        sqy = bw.tile([128, BARC], F32)
        n2 = bw.tile([128, BARC], F32)
        sr = bw.tile([128, BARC], F32)
        dd = bw.tile([128, BARC], F32)
        rr = bw.tile([128, BARC], F32)
        rpx = bw.tile([128, BARC], F32)
        rpy = bw.tile([128, BARC], F32)
        ptx = bw.tile([128, BARC // 4], F16)
        pty = bw.tile([128, BARC // 4], F16)
        barS = bw.tile([32, 1024], F32)
        xbx = xb[:, 0:BARC]
        xby = xb[:, BARC:2 * BARC]

        def bar_step(i):
            if i == 0:
                nc.gpsimd.tensor_tensor(sqx[:], xbx, xbx, op=ALU.mult)
            elif i == 1:
                nc.gpsimd.tensor_tensor(sqy[:], xby, xby, op=ALU.mult)
            elif i == 2:
                nc.gpsimd.tensor_tensor(n2[:], sqx[:], sqy[:], op=ALU.add)
            elif i == 3:
                nc.scalar.activation(sr[:], n2[:], AF.Sqrt)
            elif i == 4:
                nc.gpsimd.tensor_scalar(dd[:], sr[:], 1.0 / B_GAMMA,
                                         -DS / B_GAMMA, op0=ALU.mult, op1=ALU.add)
            elif i == 5:
                nc.vector.reciprocal_approx_fast(out=rr[:], in_=dd[:])
            elif i == 6:
                nc.gpsimd.tensor_tensor(rpx[:], xbx, rr[:], op=ALU.mult)
            elif i == 7:
                nc.gpsimd.tensor_tensor(rpy[:], xby, rr[:], op=ALU.mult)
            elif i == 8:
                nc.vector.tensor_reduce(
                    out=ptx[:], in_=rpx[:].rearrange("p (c f) -> p c f", f=4),
                    axis=mybir.AxisListType.X, op=ALU.add)
            elif i == 9:
                nc.vector.tensor_reduce(
                    out=pty[:], in_=rpy[:].rearrange("p (c f) -> p c f", f=4),
                    axis=mybir.AxisListType.X, op=ALU.add)
            elif i == 10:
                bx = p3.tile([32, 400], F32, tag="pb", name="barxp")
                nc.tensor.matmul(bx[:], lhsT=ones4[:], rhs=ptx[:],
                                 start=True, stop=True)
                nc.vector.tensor_copy(barS[:, 0:400], bx[:])
            elif i == 11:
                by = p3.tile([32, 400], F32, tag="pb", name="baryp")
                nc.tensor.matmul(by[:], lhsT=ones4[:], rhs=pty[:],
                                 start=True, stop=True)
                nc.vector.tensor_copy(barS[:, 512:912], by[:])

        peacc = [None, None]

        def chunk_tail(c):
            et = pep.tile([32, 512], F32, tag="et")
            nc.vector.transpose(et[:], peacc[c][:])
            e1 = pep.tile([32, 512], F32, tag="e1")
            nc.scalar.activation(e1[:], et[:], AF.Tanh)
            act = pep.tile([32, 512], F32, tag="act")
            actv = act[:].rearrange("p (b m u) -> p b m u", m=16, u=2)
            e1v = e1[:].rearrange("p (b m u) -> p b m u", m=16, u=2)
            bxv = barS[:, 256 * c:256 * c + 256].rearrange(
                "p (m b o) -> p b m o", b=16, o=1)
            byv = barS[:, 512 + 256 * c:512 + 256 * c + 256].rearrange(
                "p (m b o) -> p b m o", b=16, o=1)
            nc.vector.tensor_add(actv[:, :, :, 0:1], e1v[:, :, :, 0:1], bxv)
            nc.vector.tensor_add(actv[:, :, :, 1:2], e1v[:, :, :, 1:2], byv)
            yt = pep.tile([32, 512], F32, tag="yt")
            nc.scalar.activation(yt[:], act[:], AF.Tanh)
            y2 = pep.tile([32, 512], F32, tag="y2")
            nc.vector.tensor_scalar_mul(y2[:], yt[:], 2.0)
            nc.sync.dma_start(y_d[:, 512 * c:512 * c + 512], y2[:])

        # HAM warm-up: ~4us of back-to-back dummy matmuls while the input
        # DMAs stream, so the PE clock gate is at 8/8 when group 0 lands.
        for wi in range(18):
            wps = p1.tile([128, 512], F32, tag="ps", name=f"warm{wi}")
            nc.tensor.matmul(wps[:], lhsT=w1p[:, 0:128], rhs=w1p[:, 0:512],
                             start=True, stop=True)

        # 3-stage software pipeline: iteration i runs layer-1(i) interleaved
        # with the fused layer-2 DR matmuls of group i-1 and the psi head of
        # group i-2, so TensorE never waits on a just-issued evac.
        st1 = None   # group i-1 state
        st2 = None   # group i-2 state
        for i in range(G512 + 2):
            cur = None
            if i < G512:
                cs = i * 512
                xt = xin.tile([128, 512], F16, tag="xt", name=f"xt{i}")
                nc.sync.dma_start(xt[:], xt_d[:, cs:cs + 512])
                rg = rgp.tile([66, 512], F16, tag="rg", bufs=3, name=f"rg{i}")
                nc.sync.dma_start(rg[64:66, :], gg_d[:, cs:cs + 512])
                parents = [s8p.tile([128, 1024], F8, tag=f"s{j}",
                                    name=f"s8_{i}_{j}") for j in range(6)]
                cur = dict(i=i, xt=xt, rg=rg, parents=parents)

            if st2 is not None:
                # psi head for group i-2 (rh/phh evacs ran last iteration)
                g2 = st2["i"]
                m2, c2 = g2 % 16, g2 // 16
                pb = p3.tile([128, 512], F32, tag="pb", name=f"pb{g2}")
                nc.tensor.matmul(pb[0:64, :], lhsT=pw[:], rhs=st2["rg"][:],
                                 start=True, stop=True)

            prh = None
            if st1 is not None:
                prh = p2.tile([64, 512], F32, tag="prh", name=f"prh{st1['i']}")
                st1["prh"] = prh

            for j in range(6):
                if cur is not None:
                    ps = p1.tile([128, 1024], F32, tag="ps", name=f"ps_{i}_{j}")
                    for ko in range(2):
                        idx = 2 * j + ko
                        if idx < 8:
                            lhsT = w1p[:, idx * 128:idx * 128 + 128]
                        else:
                            lhsT = ow1p[:, (idx - 8) * 128:(idx - 8) * 128 + 128]
                        nc.tensor.matmul(ps[:, ko * 512:ko * 512 + 512],
                                         lhsT=lhsT, rhs=cur["xt"][:],
                                         start=True, stop=True)
                    bcol = small[:, 0:1] if j < 4 else small[:, 1:2]
                    dst = cur["parents"][j][:]
                    if j % 2 == 0:
                        nc.scalar.activation(dst, ps[:], AF.Relu, bias=bcol)
                    else:
                        nc.vector.tensor_scalar(dst, ps[:], bcol, 0.0,
                                                op0=ALU.add, op1=ALU.max)
                if st1 is not None:
                    rhs = st1["parents"][j][:].rearrange("p (ko n) -> p ko n", ko=2)
                    wsel = l2w[:, 0:128] if j < 4 else l2w[:, 128:256]
                    lhsT = wsel.rearrange("p (ko q) -> p ko q", ko=2)
                    nc.tensor.matmul(st1["prh"][:], lhsT=lhsT, rhs=rhs,
                                     start=(j == 0), stop=(j == 5),
                                     perf_mode=PM.DoubleRow)

            if st2 is not None:
                phh = rgp.tile([65, 512], F16, tag="phh", name=f"phh{g2}")
                nc.gpsimd.memset(phh[64:65, :], 1.0)
                nc.scalar.activation(phh[0:64, :], pb[0:64, :], AF.Relu,
                                     bias=small[0:64, 3:4])
                nc.tensor.matmul(pb[64:66, :], lhsT=pw2[:], rhs=phh[:],
                                 start=True, stop=True, tile_position=(0, 64))
                if m2 == 0:
                    peacc[c2] = pep.tile([32, 512], F32, tag="pe",
                                         name=f"peacc{c2}")
                petmp = rgp.tile([2, 512], F32, tag="petmp", name=f"petmp{g2}")
                nc.vector.tensor_copy(petmp[:], pb[64:66, :])
                nc.gpsimd.dma_start(peacc[c2][2 * m2:2 * m2 + 2, :], petmp[:])

            if st1 is not None:
                nc.scalar.activation(st1["rg"][0:64, :], st1["prh"][:], AF.Relu,
                                     bias=small[0:64, 2:3], scale=1.0 / (S1 * S2))

            if i < 12:
                bar_step(i)
            if st2 is not None and st2["i"] == 15:
                chunk_tail(0)
            st2, st1 = st1, cur
        chunk_tail(1)
    return nc


_CACHED = {}


def kernel(**inputs):
    x = np.asarray(inputs["x"], np.float32)
    wk = _pack_weights(**{k: np.asarray(v, np.float32) for k, v in inputs.items()
                          if k != "x"})
    in_maps = []
    for core in range(NCORE):
        xp = np.zeros((AP_, D_OBS), np.float32)
        xp[:AC] = x[core * AC:(core + 1) * AC]
        xt = np.zeros((128, AP_), np.float16)
        xt[0:D_OBS] = xp.T.astype(np.float16)
        gg = np.ascontiguousarray(xp[:, 0:2].T.astype(np.float16))
        p = -xp[:, 5:69].reshape(AP_, 16, 4)[:, :, 0:2]       # [A, 16, 2]
        # [gi, bj, a32, nhi, nlo] -> partition 32*nlo + a32, col (gi*16+bj)*4+nhi
        pr = p.reshape(G512, 16, 32, 4, 4, 2)
        xbh = pr.transpose(5, 4, 2, 0, 1, 3).reshape(2, 128, BARC)
        xb = np.ascontiguousarray(
            np.concatenate([xbh[0], xbh[1]], axis=1).astype(np.float32))
        m = dict(wk)
        m["xt"] = np.ascontiguousarray(xt)
        m["gg"] = gg
        m["xb"] = xb
        in_maps.append(m)

    if "nc" not in _CACHED:
        nc = bacc.Bacc("TRN2", target_bir_lowering=False, debug=False,
                       num_devices=NCORE)
        _build(nc)
        nc.compile()
        _CACHED["nc"] = nc
    nc = _CACHED["nc"]
    res = run_bass_kernel_spmd(nc, in_maps, core_ids=list(range(NCORE)))
    _CACHED["last_res"] = res
    out = np.empty((B, ADIM), np.float32)
    for core in range(NCORE):
        Y = res.results[core]["y"]                            # [32, 1024]
        Y5 = Y.reshape(32, 2, 16, 16, 2).transpose(1, 3, 2, 0, 4)
        Y5 = Y5.reshape(32, 512, 2)[:G512].reshape(AP_, 2)
        out[core * AC:(core + 1) * AC] = Y5[:AC]
    return out


if __name__ == "__main__":
    import reference
    ins = {k: np.asarray(v) for k, v in reference.setup_inputs().items()}
    got = kernel(**ins)
    exp = np.asarray(reference.reference(**ins))
    err = np.abs(got - exp).max()
    rel = err / np.abs(exp).max()
    print(f"absmax {err:.4e} rel {rel:.4e}")


# revision 16
# speedup vs baseline: 2.3005x; 1.0259x over previous
"""Barrier_Net TRN2 kernel: 8-core data-parallel Bass/Tile implementation (v2).

Per-core structure (12800 padded agents = 25 groups x 512):
  - Layer-1 MLPs: 12 fp16 matmuls/group (weights pre-scaled x32), relu evac
    to fp8e4 (values 32*relu) split across ScalarE/VectorE.
  - Layer-2 + rho1 fused by linearity (lhsT = (w2 @ rho_w1) * 8 in fp8),
    6 DoubleRow fp8 matmuls/group (contract 256) accumulate rho hidden
    pre-act (x256) in one PSUM tile.
  - rho2 + psi1 fused (lhsT rows 0:64 = rho_w2 @ psi_w1[:2] / 256), g rows
    appended to the same rhs tile -> single matmul.
  - empty-head e stashed per group into a [32,512] accumulator (16 groups),
    transposed once per chunk, tanh'd in batch (one act-table switch total).
  - barrier computed fully in fp32 (accuracy-critical) in a [128,1600]x2
    layout, neighbor reduce on DVE + partition fold via matmul with ones.
"""
import sys, os
sys.path.insert(0, "/opt/trn_rl_repo")
import numpy as np
import ml_dtypes
import concourse.bacc as bacc
import concourse.tile as tile
import concourse.mybir as mybir
from concourse.bass_utils import run_bass_kernel_spmd
from contextlib import ExitStack

F32 = mybir.dt.float32
F16 = mybir.dt.float16
F8 = mybir.dt.float8e4
AF = mybir.ActivationFunctionType
ALU = mybir.AluOpType
PM = mybir.MatmulPerfMode

B, NN, NO, SD = 100000, 16, 8, 4
H, PHI_OUT, ADIM = 64, 16, 2
DS, B_GAMMA = 0.2, 0.01
D_OBS = 85
NCORE = 8
AC = B // NCORE            # 12500 agents per core
G512 = 25                  # groups of 512
AP_ = G512 * 512           # padded agents per core = 12800
S1 = 32.0                  # layer-1 weight prescale
S2 = 8.0                   # fused layer-2 weight prescale
BARC = G512 * 16 * 4       # 1600 barrier cols per half

SCALAR_EVACS = frozenset({0, 1, 3, 4, 6, 7, 9, 10})


def _pack_weights(phi_w1, phi_b1, phi_w2, phi_b2, obs_w1, obs_b1, obs_w2, obs_b2,
                  rho_w1, rho_b1, rho_w2, rho_b2, psi_w1, psi_b1, psi_w2, psi_b2):
    # layer-1 block-diagonal lhsT, 2 elements per matmul, prescaled x32
    W1P = np.zeros((8, 128, 128), np.float32)
    for k in range(8):
        for j in range(2):
            n = 2 * k + j
            W1P[k, 5 + 4 * n:5 + 4 * n + 4, 64 * j:64 * j + 64] = phi_w1 * S1
    OW1P = np.zeros((4, 128, 128), np.float32)
    for m in range(4):
        for j in range(2):
            o = 2 * m + j
            OW1P[m, 69 + 2 * o:69 + 2 * o + 2, 64 * j:64 * j + 64] = obs_w1 * S1
    # fused layer-2 + rho1 (DoubleRow lhsT [128, ko=2, 64]), x8
    W2R = np.tile((phi_w2 @ rho_w1) * S2, (2, 1))       # [128, 64]
    OW2R = np.tile((obs_w2 @ rho_w1) * S2, (2, 1))
    l2w = np.zeros((128, 2, 2, 64), np.float32)          # [p, which, ko, j]
    l2w[:, 0, 0] = l2w[:, 0, 1] = W2R
    l2w[:, 1, 0] = l2w[:, 1, 1] = OW2R
    # fused rho2+psi1 lhsT [66, 64]; rh carries a 1/(S1*S2) scale already
    PW = np.zeros((66, 64), np.float32)
    PW[0:64] = rho_w2 @ psi_w1[0:2]
    PW[64:66] = psi_w1[2:4]
    # bias columns
    RB1C = (rho_b1 + (NN * phi_b2 + NO * obs_b2) @ rho_w1).reshape(64, 1)
    PB1C = (psi_b1 + rho_b2 @ psi_w1[0:2]).reshape(64, 1)
    PB2C = psi_b2.reshape(2, 1)
    small = np.zeros((128, 8), np.float32)
    small[:, 0:1] = np.tile(phi_b1, 2).reshape(128, 1) * S1
    small[:, 1:2] = np.tile(obs_b1, 2).reshape(128, 1) * S1
    small[0:64, 2:3] = RB1C
    small[0:64, 3:4] = PB1C
    small[0:2, 4:5] = PB2C
    ones4 = np.zeros((128, 32), np.float32)
    for p in range(128):
        ones4[p, p % 32] = 1.0
    f8 = ml_dtypes.float8_e4m3
    PW2B = np.concatenate([psi_w2, psi_b2.reshape(1, 2)], axis=0)
    return {
        "w1p": W1P.transpose(1, 0, 2).reshape(128, 8 * 128).astype(np.float16),
        "ow1p": OW1P.transpose(1, 0, 2).reshape(128, 4 * 128).astype(np.float16),
        "l2w": l2w.reshape(128, 256).astype(f8),
        "pw": PW.astype(np.float16),
        "pw2": PW2B.astype(np.float16),
        "ones4": ones4.astype(np.float16),
        "small": small.astype(np.float32),
    }


def _build(nc):
    xt_d = nc.dram_tensor("xt", [128, AP_], F16, kind="ExternalInput").ap()
    gg_d = nc.dram_tensor("gg", [2, AP_], F16, kind="ExternalInput").ap()
    xb_d = nc.dram_tensor("xb", [128, 2 * BARC], F32, kind="ExternalInput").ap()
    w1p_d = nc.dram_tensor("w1p", [128, 8 * 128], F16, kind="ExternalInput").ap()
    ow1p_d = nc.dram_tensor("ow1p", [128, 4 * 128], F16, kind="ExternalInput").ap()
    l2w_d = nc.dram_tensor("l2w", [128, 256], F8, kind="ExternalInput").ap()
    pw_d = nc.dram_tensor("pw", [66, 64], F16, kind="ExternalInput").ap()
    pw2_d = nc.dram_tensor("pw2", [65, 2], F16, kind="ExternalInput").ap()
    ones4_d = nc.dram_tensor("ones4", [128, 32], F16, kind="ExternalInput").ap()
    small_d = nc.dram_tensor("small", [128, 8], F32, kind="ExternalInput").ap()
    y_d = nc.dram_tensor("y", [32, 1024], F32, kind="ExternalOutput").ap()

    with tile.TileContext(nc) as tc, ExitStack() as ctx, \
            nc.allow_low_precision(reason="fp16 barrier partials validated"):
        cw = ctx.enter_context(tc.tile_pool(name="cw", bufs=1))
        xin = ctx.enter_context(tc.tile_pool(name="xin", bufs=3))
        s8p = ctx.enter_context(tc.tile_pool(name="s8p", bufs=2))
        rgp = ctx.enter_context(tc.tile_pool(name="rgp", bufs=2))
        pep = ctx.enter_context(tc.tile_pool(name="pep", bufs=2))
        bw = ctx.enter_context(tc.tile_pool(name="bw", bufs=1))
        p1 = ctx.enter_context(tc.tile_pool(name="p1", bufs=3, space="PSUM"))
        p2 = ctx.enter_context(tc.tile_pool(name="p2", bufs=1, space="PSUM"))
        p3 = ctx.enter_context(tc.tile_pool(name="p3", bufs=1, space="PSUM"))

        w1p = cw.tile([128, 8 * 128], F16); nc.sync.dma_start(w1p[:], w1p_d)
        ow1p = cw.tile([128, 4 * 128], F16); nc.sync.dma_start(ow1p[:], ow1p_d)
        l2w = cw.tile([128, 256], F8); nc.sync.dma_start(l2w[:], l2w_d)
        pw = cw.tile([66, 64], F16); nc.sync.dma_start(pw[:], pw_d)
        pw2 = cw.tile([65, 2], F16); nc.sync.dma_start(pw2[:], pw2_d)
        ones4 = cw.tile([128, 32], F16); nc.sync.dma_start(ones4[:], ones4_d)
        small = cw.tile([128, 8], F32); nc.sync.dma_start(small[:], small_d)
        xb = cw.tile([128, 2 * BARC], F32)

        # barrier working tiles (fp32 until the neighbor reduce)
        sqx = bw.tile([128, BARC], F32)
        sqy = bw.tile([128, BARC], F32)
        n2 = bw.tile([128, BARC], F32)
        sr = bw.tile([128, BARC], F32)
        dd = bw.tile([128, BARC], F32)
        rr = bw.tile([128, BARC], F32)
        rpx = bw.tile([128, BARC], F32)
        rpy = bw.tile([128, BARC], F32)
        ptx = bw.tile([128, BARC // 4], F16)
        pty = bw.tile([128, BARC // 4], F16)
        barS = bw.tile([32, 1024], F32)
        xbx = xb[:, 0:BARC]
        xby = xb[:, BARC:2 * BARC]

        def bar_step(i):
            if i == 0:
                nc.gpsimd.tensor_tensor(sqx[:], xbx, xbx, op=ALU.mult)
            elif i == 1:
                nc.gpsimd.tensor_tensor(sqy[:], xby, xby, op=ALU.mult)
            elif i == 2:
                nc.gpsimd.tensor_tensor(n2[:], sqx[:], sqy[:], op=ALU.add)
            elif i == 3:
                nc.scalar.activation(sr[:], n2[:], AF.Sqrt)
            elif i == 4:
                nc.gpsimd.tensor_scalar(dd[:], sr[:], 1.0 / B_GAMMA,
                                         -DS / B_GAMMA, op0=ALU.mult, op1=ALU.add)
            elif i == 5:
                nc.vector.reciprocal_approx_fast(out=rr[:], in_=dd[:])
            elif i == 6:
                nc.gpsimd.tensor_tensor(rpx[:], xbx, rr[:], op=ALU.mult)
            elif i == 7:
                nc.gpsimd.tensor_tensor(rpy[:], xby, rr[:], op=ALU.mult)
            elif i == 8:
                nc.vector.tensor_reduce(
                    out=ptx[:], in_=rpx[:].rearrange("p (c f) -> p c f", f=4),
                    axis=mybir.AxisListType.X, op=ALU.add)
            elif i == 9:
                nc.vector.tensor_reduce(
                    out=pty[:], in_=rpy[:].rearrange("p (c f) -> p c f", f=4),
                    axis=mybir.AxisListType.X, op=ALU.add)
            elif i == 10:
                bx = p3.tile([32, 400], F32, tag="pb", name="barxp")
                nc.tensor.matmul(bx[:], lhsT=ones4[:], rhs=ptx[:],
                                 start=True, stop=True)
                nc.vector.tensor_copy(barS[:, 0:400], bx[:])
            elif i == 11:
                by = p3.tile([32, 400], F32, tag="pb", name="baryp")
                nc.tensor.matmul(by[:], lhsT=ones4[:], rhs=pty[:],
                                 start=True, stop=True)
                nc.vector.tensor_copy(barS[:, 512:912], by[:])

        peacc = [None, None]

        def chunk_tail(c):
            et = pep.tile([32, 512], F32, tag="et")
            nc.vector.transpose(et[:], peacc[c][:])
            e1 = pep.tile([32, 512], F32, tag="e1")
            nc.scalar.activation(e1[:], et[:], AF.Tanh)
            act = pep.tile([32, 512], F32, tag="act")
            actv = act[:].rearrange("p (b m u) -> p b m u", m=16, u=2)
            e1v = e1[:].rearrange("p (b m u) -> p b m u", m=16, u=2)
            bxv = barS[:, 256 * c:256 * c + 256].rearrange(
                "p (m b o) -> p b m o", b=16, o=1)
            byv = barS[:, 512 + 256 * c:512 + 256 * c + 256].rearrange(
                "p (m b o) -> p b m o", b=16, o=1)
            nc.vector.tensor_add(actv[:, :, :, 0:1], e1v[:, :, :, 0:1], bxv)
            nc.vector.tensor_add(actv[:, :, :, 1:2], e1v[:, :, :, 1:2], byv)
            yt = pep.tile([32, 512], F32, tag="yt")
            nc.scalar.activation(yt[:], act[:], AF.Tanh)
            y2 = pep.tile([32, 512], F32, tag="y2")
            nc.vector.tensor_scalar_mul(y2[:], yt[:], 2.0)
            nc.sync.dma_start(y_d[:, 512 * c:512 * c + 512], y2[:])

        # HAM warm-up: ~4us of back-to-back dummy matmuls while the input
        # DMAs stream, so the PE clock gate is at 8/8 when group 0 lands.
        for wi in range(18):
            wps = p1.tile([128, 512], F32, tag="ps", name=f"warm{wi}")
            nc.tensor.matmul(wps[:], lhsT=w1p[:, 0:128], rhs=w1p[:, 0:512],
                             start=True, stop=True)

        # 3-stage software pipeline: iteration i runs layer-1(i) interleaved
        # with the fused layer-2 DR matmuls of group i-1 and the psi head of
        # group i-2, so TensorE never waits on a just-issued evac.
        st1 = None   # group i-1 state
        st2 = None   # group i-2 state
        for i in range(G512 + 2):
            cur = None
            if i == 2:
                nc.sync.dma_start(xb[:], xb_d)
            if i < G512:
                cs = i * 512
                xt = xin.tile([128, 512], F16, tag="xt", name=f"xt{i}")
                nc.sync.dma_start(xt[:], xt_d[:, cs:cs + 512])
                rg = rgp.tile([66, 512], F16, tag="rg", bufs=3, name=f"rg{i}")
                nc.sync.dma_start(rg[64:66, :], gg_d[:, cs:cs + 512])
                parents = [s8p.tile([128, 1024], F8, tag=f"s{j}",
                                    name=f"s8_{i}_{j}") for j in range(6)]
                cur = dict(i=i, xt=xt, rg=rg, parents=parents)

            if st2 is not None:
                # psi head for group i-2 (rh/phh evacs ran last iteration)
                g2 = st2["i"]
                m2, c2 = g2 % 16, g2 // 16
                pb = p3.tile([128, 512], F32, tag="pb", name=f"pb{g2}")
                nc.tensor.matmul(pb[0:64, :], lhsT=pw[:], rhs=st2["rg"][:],
                                 start=True, stop=True)

            prh = None
            if st1 is not None:
                prh = p2.tile([64, 512], F32, tag="prh", name=f"prh{st1['i']}")
                st1["prh"] = prh

            for j in range(6):
                if cur is not None:
                    ps = p1.tile([128, 1024], F32, tag="ps", name=f"ps_{i}_{j}")
                    for ko in range(2):
                        idx = 2 * j + ko
                        if idx < 8:
                            lhsT = w1p[:, idx * 128:idx * 128 + 128]
                        else:
                            lhsT = ow1p[:, (idx - 8) * 128:(idx - 8) * 128 + 128]
                        nc.tensor.matmul(ps[:, ko * 512:ko * 512 + 512],
                                         lhsT=lhsT, rhs=cur["xt"][:],
                                         start=True, stop=True)
                    bcol = small[:, 0:1] if j < 4 else small[:, 1:2]
                    dst = cur["parents"][j][:]
                    if j % 2 == 0:
                        nc.scalar.activation(dst, ps[:], AF.Relu, bias=bcol)
                    else:
                        nc.vector.tensor_scalar(dst, ps[:], bcol, 0.0,
                                                op0=ALU.add, op1=ALU.max)
                if st1 is not None:
                    rhs = st1["parents"][j][:].rearrange("p (ko n) -> p ko n", ko=2)
                    wsel = l2w[:, 0:128] if j < 4 else l2w[:, 128:256]
                    lhsT = wsel.rearrange("p (ko q) -> p ko q", ko=2)
                    nc.tensor.matmul(st1["prh"][:], lhsT=lhsT, rhs=rhs,
                                     start=(j == 0), stop=(j == 5),
                                     perf_mode=PM.DoubleRow)

            if st2 is not None:
                phh = rgp.tile([65, 512], F16, tag="phh", name=f"phh{g2}")
                nc.gpsimd.memset(phh[64:65, :], 1.0)
                nc.scalar.activation(phh[0:64, :], pb[0:64, :], AF.Relu,
                                     bias=small[0:64, 3:4])
                nc.tensor.matmul(pb[64:66, :], lhsT=pw2[:], rhs=phh[:],
                                 start=True, stop=True, tile_position=(0, 64))
                if m2 == 0:
                    peacc[c2] = pep.tile([32, 512], F32, tag="pe",
                                         name=f"peacc{c2}")
                petmp = rgp.tile([2, 512], F32, tag="petmp", name=f"petmp{g2}")
                nc.vector.tensor_copy(petmp[:], pb[64:66, :])
                nc.sync.dma_start(peacc[c2][2 * m2:2 * m2 + 2, :], petmp[:])

            if st1 is not None:
                nc.scalar.activation(st1["rg"][0:64, :], st1["prh"][:], AF.Relu,
                                     bias=small[0:64, 2:3], scale=1.0 / (S1 * S2))

            if i < 12:
                bar_step(i)
            if st2 is not None and st2["i"] == 15:
                chunk_tail(0)
            st2, st1 = st1, cur
        chunk_tail(1)
    return nc


_CACHED = {}


def kernel(**inputs):
    x = np.asarray(inputs["x"], np.float32)
    wk = _pack_weights(**{k: np.asarray(v, np.float32) for k, v in inputs.items()
                          if k != "x"})
    in_maps = []
    for core in range(NCORE):
        xp = np.zeros((AP_, D_OBS), np.float32)
        xp[:AC] = x[core * AC:(core + 1) * AC]
        xt = np.zeros((128, AP_), np.float16)
        xt[0:D_OBS] = xp.T.astype(np.float16)
        gg = np.ascontiguousarray(xp[:, 0:2].T.astype(np.float16))
        p = -xp[:, 5:69].reshape(AP_, 16, 4)[:, :, 0:2]       # [A, 16, 2]
        # [gi, bj, a32, nhi, nlo] -> partition 32*nlo + a32, col (gi*16+bj)*4+nhi
        pr = p.reshape(G512, 16, 32, 4, 4, 2)
        xbh = pr.transpose(5, 4, 2, 0, 1, 3).reshape(2, 128, BARC)
        xb = np.ascontiguousarray(
            np.concatenate([xbh[0], xbh[1]], axis=1).astype(np.float32))
        m = dict(wk)
        m["xt"] = np.ascontiguousarray(xt)
        m["gg"] = gg
        m["xb"] = xb
        in_maps.append(m)

    if "nc" not in _CACHED:
        nc = bacc.Bacc("TRN2", target_bir_lowering=False, debug=False,
                       num_devices=NCORE)
        _build(nc)
        nc.compile()
        _CACHED["nc"] = nc
    nc = _CACHED["nc"]
    res = run_bass_kernel_spmd(nc, in_maps, core_ids=list(range(NCORE)))
    _CACHED["last_res"] = res
    out = np.empty((B, ADIM), np.float32)
    for core in range(NCORE):
        Y = res.results[core]["y"]                            # [32, 1024]
        Y5 = Y.reshape(32, 2, 16, 16, 2).transpose(1, 3, 2, 0, 4)
        Y5 = Y5.reshape(32, 512, 2)[:G512].reshape(AP_, 2)
        out[core * AC:(core + 1) * AC] = Y5[:AC]
    return out


if __name__ == "__main__":
    import reference
    ins = {k: np.asarray(v) for k, v in reference.setup_inputs().items()}
    got = kernel(**ins)
    exp = np.asarray(reference.reference(**ins))
    err = np.abs(got - exp).max()
    rel = err / np.abs(exp).max()
    print(f"absmax {err:.4e} rel {rel:.4e}")
